# revision 23
# baseline (speedup 1.0000x reference)
"""TRN2 Bass kernel for nn_CIFAR10_Type1_Template_Unroll (dense_cnn).

Network (per reference): two locally-connected conv layers + 3-layer FC
head, B=4096. Strategy: pure data parallel over 8 NeuronCores (512 batch
each), activations kept on-chip in [feature, batch] layout, batch N=512
on the matmul free dim throughout.

v5 design notes (v3 measured 146.4us; v4's two-queue split halved the
x stream via per-packet round-robin, starved the PE, re-throttled HAM
to 1.2GHz for 14us -> 148us. v5 reverts to ONE queue):
- Everything fp16 (inputs, weights, activations; PSUM accumulate fp32).
  Measured end-to-end error ~1e-3 vs the 2e-2 gate. Halves DMA bytes.
- ALL weight/input DMAs ride the single sync HWDGE queue in EXACT
  first-need order: x r0, w1(rows 0-3), x r1-r3, w2 pass-0 pairs, x r4,
  w1(4-9), ... , w2 pass6, fc2, fc3, fc1 m0..m7, y-out. A single FIFO
  queue gives the head transfer the full ~390 GB/s; queue position IS
  the pacing policy (pool-buffer WAR stalls at the queue head only ever
  delay strictly-later-needed transfers). No SWDGE, no gate tricks.
- x streams as 16 single-row [128,1024] tiles (256KB completion
  granularity) so row 0 lands ~10us; xp pool bufs=10 so x WAR stalls
  can't block the early w2/w1 transfers behind them.
- PE warmup: 6 full-array K=128 matmuls on a memset tile (~2.6us at
  the cold 1.2GHz clock) + 1 filler after each of rows 0-2; the HAM
  un-throttle window then lands during row 0-2 work. Any PE gap >~3.4us
  re-throttles the clock (v4's failure) -- the schedule keeps gaps
  under ~1.5us.
- L1 PSUM evac was v3's hidden bottleneck: one-engine [128,1024] evacs
  (~1.1us) gated the 2-buffer PSUM recycle, stretching L1 from 6.8us
  ideal to ~18us measured. v5 splits every L1 evac across ACT (cols
  0:512, bank A) + DVE (cols 512:1024, bank B) -- legal, different
  banks, verified overlapping in the v4 trace -- and deepens l1ps to
  bufs=3 (6 banks; l2ps gets 2).
- L2 emitted at full-chain granularity (8 k-steps, one pair of
  positions on PE col halves 0-63/64-127), interleaved with the L1
  half-rows each pass unlocks: pass h = [chain, half-row, chain, ...].
- FC head unchanged in math; fcps bufs=4 + fc1 weights prefetched from
  the sync queue (bufs=3, WAR self-paced) remove the inter-chain
  stalls v3/v4 showed. FC3 interleaves into the FC2 chain loop.
"""
import sys

if '/opt/trn_rl_repo' not in sys.path:
    sys.path.insert(0, '/opt/trn_rl_repo')

import numpy as np

N_CORES = 8
BS = 512
WARM_N = 12
LAST_EXEC_NS = None

# ----------------------------------------------------------------- host prep

def _prep_x(x):
    """x [B,3,32,32] -> [N_CORES, 16, 128, 1024] f16 row tiles.

    part = 32*i + 16*q + f; pair p=4g+i covers w1 in {2p,2p+1}; q = w1
    parity; f = c*4 + kh*2 + kw (12..15 zero-pad). Free dim = (g, batch).
    """
    ncr = x.shape[0] // BS
    xr = x.reshape(ncr, BS, 3, 16, 2, 2, 4, 2, 2)   # s,b,c,r,kh,g,i,q,kw
    xt = xr.transpose(0, 3, 5, 6, 7, 2, 4, 8, 1)    # s,r,g,i,q,c,kh,kw,b
    xt = xt.reshape(ncr, 16, 2, 4, 2, 12, BS)
    xpp = np.zeros((ncr, 16, 2, 4, 2, 16, BS), np.float16)
    xpp[..., :12, :] = xt
    # -> s, r, (i,q,f)=128, (g,b)=1024
    xpp = xpp.reshape(ncr, 16, 2, 128, BS).transpose(0, 1, 3, 2, 4)
    return np.ascontiguousarray(xpp.reshape(ncr, 16, 128, 1024))


def _prep_w1(conv1w):
    """conv1w [64,256,3,2,2] -> [128, 16*256] f16 block-diag strips.

    [p, r*256 + g*128 + c]: strip part p = 32i+16qp+f holds, for parity
    qp, features f -> out channel block c = 64*q + o with q==qp.
    """
    w1r = conv1w.reshape(64, 16, 16, 3, 2, 2)
    wt = w1r.transpose(1, 2, 3, 4, 5, 0).reshape(16, 16, 12, 64)
    wtp = np.zeros((16, 16, 16, 64), np.float32)
    wtp[:, :, :12, :] = wt
    wtp = wtp.reshape(16, 2, 4, 2, 16, 64)          # r,g,i,qp,f,o
    w1t = np.zeros((16, 2, 4, 2, 16, 2, 64), np.float32)
    w1t[:, :, :, 0, :, 0, :] = wtp[:, :, :, 0, :, :]
    w1t[:, :, :, 1, :, 1, :] = wtp[:, :, :, 1, :, :]
    w1t = w1t.reshape(16, 2, 128, 128)              # r,g,p,c
    w1t = w1t.transpose(2, 0, 1, 3)                 # p,r,g,c
    full = w1t.reshape(128, 16 * 256).astype(np.float16)
    # three DRAM-contiguous chunks in need order (rows 0-3 / 4-9 / 10-15):
    # a strided [:, lo:hi] slice of one [128,4096] tensor drains at
    # ~75-100 GB/s (2KB pieces at 8KB stride) and stalls the FIFO queue.
    return (np.ascontiguousarray(full[:, 0:1024]),
            np.ascontiguousarray(full[:, 1024:2560]),
            np.ascontiguousarray(full[:, 2560:4096]))


def _h2_posmap():
    pm = np.full((25, 2), -1, np.int64)
    for T in range(21):
        rr, j = divmod(T, 3)
        pm[T, 0] = rr * 7 + 2 * j
        pm[T, 1] = rr * 7 + 2 * j + 1
    for pi in range(4):
        r0, r1 = 2 * pi, 2 * pi + 1
        pm[21 + pi, 0] = r0 * 7 + 6
        if r1 < 7:
            pm[21 + pi, 1] = r1 * 7 + 6
    return pm


# pair-tile consumption order: pass h emits pairs [3h, 3h+1, 3h+2] plus
# cross pairs 21/22/23+24 at passes 2/4/6; w2 DRAM tiles are stored in
# this exact order so each pass is one contiguous DMA.
_W2_ORDER = [0, 1, 2, 3, 4, 5, 6, 7, 8, 21, 9, 10, 11, 12, 13, 14, 22,
             15, 16, 17, 18, 19, 20, 23, 24]
_W2_SLOT = {T: s for s, T in enumerate(_W2_ORDER)}


def _prep_w2(conv2w):
    """conv2w [64,49,64,4,4] -> [25, 128, 1024] f16 pair tiles in
    consumption (_W2_ORDER) order.

    Per position: [128=(q,c), 512=(kh,t,o)]; pair tile free dim =
    (member u, 512).
    """
    w2r = conv2w.reshape(64, 7, 7, 64, 4, 4)
    v = w2r.transpose(1, 2, 3, 4, 5, 0)             # h,w,c,kh,kw,o
    v = v.reshape(7, 7, 64, 4, 2, 2, 64)            # h,w,c,kh,t,q,o
    v = v.transpose(0, 1, 5, 2, 3, 4, 6)            # h,w,q,c,kh,t,o
    pos = v.reshape(49, 128, 512)
    pm = _h2_posmap()
    out = np.zeros((25, 128, 1024), np.float16)
    for T in range(25):
        s = _W2_SLOT[T]
        out[s, :, 0:512] = pos[pm[T, 0]]
        if pm[T, 1] >= 0:
            out[s, :, 512:1024] = pos[pm[T, 1]]
    return np.ascontiguousarray(out)


def _prep_fc1(fc1):
    """fc1 [1024, 3136] -> [8, 128, 3200] f16, k in h2-tile (T) order."""
    pm = _h2_posmap()
    fc1p = fc1.reshape(1024, 64, 49)
    fc1hat = np.zeros((1024, 25, 2, 64), np.float32)
    for T in range(25):
        for u in range(2):
            p = pm[T, u]
            if p >= 0:
                fc1hat[:, T, u, :] = fc1p[:, :, p]
    a = fc1hat.reshape(8, 128, 25, 128).transpose(0, 3, 2, 1)   # m,kp,k,mc
    return np.ascontiguousarray(a.reshape(8, 128, 3200)).astype(np.float16)


def _prep_fc2(fc2):
    """fc2 [512, 1024] -> [128, 4096] f16: [kp, (m k mc)]."""
    a = fc2.reshape(4, 128, 8, 128)                 # m,mc,k,kp
    a = a.transpose(3, 0, 2, 1)                     # kp,m,k,mc
    return np.ascontiguousarray(a.reshape(128, 4096)).astype(np.float16)


def _prep_fc3(fc3):
    """fc3 [10, 512] -> [128, 40] f16: [kp, (k o)]."""
    a = fc3.T.reshape(4, 128, 10)                   # k,kp,o
    a = a.transpose(1, 0, 2)                        # kp,k,o
    return np.ascontiguousarray(a.reshape(128, 40)).astype(np.float16)


# --------------------------------------------------------------- bass kernel

_NC_CACHE = []


def _build_nc():
    import concourse.bass as bass
    import concourse.mybir as mybir
    from concourse import bacc
    from concourse.tile import TileContext

    f32 = mybir.dt.float32
    f16 = mybir.dt.float16
    RELU = mybir.ActivationFunctionType.Relu

    nc = bacc.Bacc("TRN2", target_bir_lowering=False, debug=False,
                   num_devices=N_CORES)
    x_pp = nc.dram_tensor("x_pp", [16, 128, 1024], f16, kind="ExternalInput")
    w1a = nc.dram_tensor("w1a", [128, 1024], f16, kind="ExternalInput")
    w1b = nc.dram_tensor("w1b", [128, 1536], f16, kind="ExternalInput")
    w1c = nc.dram_tensor("w1c", [128, 1536], f16, kind="ExternalInput")
    w2t = nc.dram_tensor("w2t", [25, 128, 1024], f16, kind="ExternalInput")
    fc1m = nc.dram_tensor("fc1m", [8, 128, 3200], f16, kind="ExternalInput")
    fc2t = nc.dram_tensor("fc2t", [128, 4096], f16, kind="ExternalInput")
    fc3t = nc.dram_tensor("fc3t", [128, 40], f16, kind="ExternalInput")
    y = nc.dram_tensor("y", [10, 512], f32, kind="ExternalOutput")

    pm = _h2_posmap()
    # pass h -> pair tiles, in chain order
    pass_pairs = {h: [3 * h + j for j in range(3)] for h in range(7)}
    pass_pairs[2].append(21)
    pass_pairs[4].append(22)
    pass_pairs[6].extend([23, 24])

    ectr = [0]

    with TileContext(nc) as tc:
        with (
            tc.tile_pool(name="h2pool", bufs=25) as h2pool,
            tc.tile_pool(name="wpool", bufs=4) as wpool,
        ):
            h2 = [h2pool.tile([128, 512], f16, tag="h2", name=f"h2_{T}")
                  for T in range(25)]

            def relu_evac(dst, src):
                if ectr[0] % 2 == 0:
                    nc.scalar.activation(dst, src, RELU)
                else:
                    nc.vector.tensor_scalar_max(dst, src, 0.0)
                ectr[0] += 1

            # The warm-tile memset rides GpSimd: that queue carries
            # nothing else, so the tile is ready the moment the engine
            # preambles finish and the first warm matmul can issue.
            warm = wpool.tile([128, 512], f16, tag="warm", name="warm",
                              bufs=1)
            nc.gpsimd.memset(warm[:], 0.0)

            # fc1 weights stream from the sync queue tail (after all
            # phase-1 transfers); bufs=3 WAR stalls at the queue head
            # self-pace m3..m7 behind the FC1 chains that free their
            # buffers. Nothing later on the queue except y-out.
            fc1w = [None] * 8

            def load_fc1(m):
                wt = wpool.tile([128, 3200], f16, tag="fc1w",
                                name=f"fc1w_{m}", bufs=3)
                nc.sync.dma_start(out=wt[:], in_=fc1m.ap()[m])
                fc1w[m] = wt

            # ---------------- phase 1: L1 + L2 interleaved ----------------
            with (
                tc.tile_pool(name="xp", bufs=10) as xp_pool,
                tc.tile_pool(name="w1p", bufs=1) as w1_pool,
                tc.tile_pool(name="w2p", bufs=3) as w2_pool,
                tc.tile_pool(name="o1p", bufs=32) as o1_pool,
                tc.tile_pool(name="l1ps", bufs=2, space="PSUM") as l1ps,
                tc.tile_pool(name="l2ps", bufs=4, space="PSUM") as l2ps,
            ):
                xt = [None] * 16

                def load_x(r):
                    t = xp_pool.tile([128, 1024], f16, tag="xp",
                                     name=f"xp_{r}")
                    nc.sync.dma_start(out=t[:], in_=x_pp.ap()[r])
                    xt[r] = t

                w1tile = w1_pool.tile([128, 4096], f16, tag="w1",
                                      name="w1")

                _w1chunks = [(w1a, 0, 1024), (w1b, 1024, 2560),
                             (w1c, 2560, 4096)]

                def load_w1(ci):
                    src, lo, hi = _w1chunks[ci]
                    nc.sync.dma_start(out=w1tile[:, lo:hi],
                                      in_=src.ap())

                # w2 tiles: pass 0 as per-pair 256KB DMAs (fine-grain
                # early arrival), passes 1-6 as whole-pass DMAs.
                w2tiles = {}

                def load_w2_pair(T, w2tl, j):
                    s = _W2_SLOT[T]
                    nc.sync.dma_start(
                        out=w2tl[:, 1024 * j:1024 * j + 1024],
                        in_=w2t.ap()[s])
                    w2tiles[T] = w2tl[:, 1024 * j:1024 * j + 1024]

                def alloc_w2(h):
                    n = len(pass_pairs[h])
                    return w2_pool.tile([128, 1024 * n], f16, tag="w2",
                                        name=f"w2p_{h}", bufs=3)

                def load_w2_pass(h):
                    # per-pair 256KB DMAs: each is DRAM-contiguous; a
                    # whole-pass gathered transfer (2KB pieces at 256KB
                    # stride) drains ~4x slower and stalls the queue.
                    t = alloc_w2(h)
                    for j, T in enumerate(pass_pairs[h]):
                        load_w2_pair(T, t, j)

                # ---- sync-queue DMA emission, exact first-need order.
                # Only fresh pool buffers here (10 x tiles, 3 w2 slots,
                # the single w1 tile); recycled buffers are emitted
                # later, after their previous tenant's readers, so the
                # emission-order WAR tracking stays sound.
                load_x(0)
                load_w1(0)                  # rows 0-3
                load_x(1)
                load_x(2)
                w2t0 = alloc_w2(0)
                load_w2_pair(0, w2t0, 0)
                load_x(3)
                load_w2_pair(1, w2t0, 1)
                load_w2_pair(2, w2t0, 2)
                load_x(4)
                load_w1(1)                  # rows 4-9
                load_x(5)
                w2t1 = alloc_w2(1)
                load_w2_pair(3, w2t1, 0)
                load_x(6)
                load_w2_pair(4, w2t1, 1)
                load_x(7)
                load_w2_pair(5, w2t1, 2)
                load_x(8)
                load_w1(2)                  # rows 10-15
                load_x(9)
                load_w2_pass(2)

                # PE warmup: full-array (K=128, M=128) matmuls on the
                # memset tile so HAM un-throttles during the DMA ramp.
                wps = l2ps.tile([128, 512], f32, tag="l2", name="warm_ps")

                def emit_warm(n):
                    for _ in range(n):
                        nc.tensor.matmul(wps[:], warm[:, 0:128], warm[:],
                                         start=True, stop=True)

                out1 = [[None] * 8 for _ in range(16)]

                def emit_l1_half_unpinned(r, g):
                    # 2 half-groups of 2 concurrent K=32 strip matmuls
                    # (tile_position row groups); each (g, half) lands
                    # in a [128,1024] PSUM tile whose evac is split
                    # ACT (cols 0:512, bank A) / DVE (512:1024, bank B)
                    # to halve the recycle latency.
                    w1row = w1tile[:, 256 * r:256 * r + 256]
                    for half in range(2):
                        ps = l1ps.tile([128, 1024], f32, tag="l1",
                                       name=f"l1ps_{r}_{g}_{half}")
                        for sub in range(2):
                            i = half + 2 * sub
                            nc.tensor.matmul(
                                ps[:, 512 * sub:512 * sub + 512],
                                w1row[32 * i:32 * i + 32,
                                      128 * g:128 * g + 128],
                                xt[r][32 * i:32 * i + 32,
                                      512 * g:512 * g + 512],
                                start=True, stop=True,
                                tile_position=(32 * i, 0))
                        ot = o1_pool.tile([128, 1024], f16, tag="o1",
                                          name=f"o1_{r}_{g}_{half}")
                        nc.scalar.activation(ot[:, 0:512],
                                             ps[:, 0:512], RELU)
                        nc.vector.tensor_scalar_max(ot[:, 512:1024],
                                                    ps[:, 512:1024],
                                                    0.0)
                        for sub in range(2):
                            out1[r][4 * g + half + 2 * sub] = \
                                ot[:, 512 * sub:512 * sub + 512]

                def emit_l1_half(r, g):
                    with tick(0.00045):
                        emit_l1_half_unpinned(r, g)

                def emit_l1_row(r):
                    emit_l1_half(r, 0)
                    emit_l1_half(r, 1)

                def emit_chain_part(T, k0, k1, ps):
                    # One position pair: A/B chains on PE col strips
                    # 0-63 / 64-127 share one [128,512] PSUM tile split
                    # by partition range; one evac covers the pair.
                    pA, pB = pm[T]
                    hA, wA = divmod(int(pA), 7)
                    hB, wB = (None, None) if pB < 0 else divmod(int(pB), 7)
                    wt2 = w2tiles[T]
                    if ps is None:
                        ps = l2ps.tile([128, 512], f32, tag="l2",
                                       name=f"l2ps_{T}")
                    for kt in range(k0, k1):
                        kh, t = divmod(kt, 2)
                        nc.tensor.matmul(
                            ps[0:64, :],
                            wt2[:, 64 * kt:64 * kt + 64],
                            out1[2 * hA + kh][wA + t],
                            start=(kt == 0), stop=(kt == 7),
                            tile_position=(0, 0))
                        if hB is not None:
                            nc.tensor.matmul(
                                ps[64:128, :],
                                wt2[:, 512 + 64 * kt:512 + 64 * kt + 64],
                                out1[2 * hB + kh][wB + t],
                                start=(kt == 0), stop=(kt == 7),
                                tile_position=(0, 64))
                    if k1 == 8:
                        relu_evac(h2[T][:], ps[:])
                    return ps

                def emit_chain_part_pinned(T, k0, k1, ps):
                    with tick(0.0019 * (k1 - k0) / 8):
                        return emit_chain_part(T, k0, k1, ps)

                def emit_chain(T):
                    emit_chain_part_pinned(T, 0, 8, None)

                def run_pass(h, rows):
                    # interleave at half-row granularity: a chain (or
                    # half-chain) between any two L1 half-rows gives the
                    # l1ps recycle (gated by the laggy DVE evac queue)
                    # enough slack. When a pass has more half-rows than
                    # chains, SPLIT trailing chains into two 4-step
                    # chunks -- two back-to-back half-rows stall ~1-2us
                    # on the PSUM WAR (v6 trace), a chunk boundary only
                    # ~0.1us.
                    ts = pass_pairs[h]
                    halves = [(r, g) for r in rows for g in range(2)]
                    n_extra = max(0, len(halves) - len(ts))
                    hi = 0

                    def next_half():
                        nonlocal hi
                        if hi < len(halves):
                            emit_l1_half(*halves[hi])
                            hi += 1

                    for j, T in enumerate(ts):
                        if j >= len(ts) - n_extra:
                            ps = emit_chain_part_pinned(T, 0, 4, None)
                            next_half()
                            emit_chain_part_pinned(T, 4, 8, ps)
                        else:
                            emit_chain(T)
                        next_half()

                # ---- PE emission: warmups + rows 0-3, then passes.
                # Recycled-buffer DMAs are emitted at the first point
                # after their WAR predecessor's readers; their queue
                # position still honors first-need order.
                #
                # Phase-1 PE units are PINNED to a paced logical clock
                # (tile_wait_until gates only the Tile scheduler's sim,
                # not runtime): without pins the scheduler weaves L1
                # half-rows INTO chain-round gaps and even splits the
                # A/B col-pair of one k-step (v7 trace, 15-21us: ~320ns
                # rounds from per-switch weight-reload exposure).
                # NOTE: forcing strict unit order via start-floor pins
                # measured 6us SLOWER (v9) -- the scheduler's fine
                # interleave works around genuinely-late dependencies
                # (DVE evac halves, DMA arrivals) better than a rigid
                # order. Keep pins as inert end-floors (no-ops in
                # practice; v8 == v7 within noise).
                pin = [0.004]

                def tick(d):
                    pin[0] += d
                    return tc.tile_wait_until(pin[0])

                emit_warm(WARM_N)
                emit_l1_row(0)
                load_x(10)
                emit_warm(2)
                emit_l1_row(1)
                load_x(11)
                emit_warm(2)
                emit_l1_row(2)
                load_x(12)
                emit_warm(1)
                emit_l1_row(3)
                load_x(13)
                emit_warm(1)
                run_pass(0, [4, 5])
                load_x(14)
                load_x(15)
                load_w2_pass(3)
                run_pass(1, [6, 7])
                load_w2_pass(4)
                run_pass(2, [8, 9])
                load_w2_pass(5)
                run_pass(3, [10, 11])
                load_w2_pass(6)
                fc2w = wpool.tile([128, 4096], f16, tag="fc2w",
                                  name="fc2w", bufs=1)
                nc.sync.dma_start(out=fc2w[:], in_=fc2t.ap())
                fc3w = wpool.tile([128, 40], f16, tag="fc3w",
                                  name="fc3w", bufs=1)
                nc.sync.dma_start(out=fc3w[:], in_=fc3t.ap())
                load_fc1(0)
                load_fc1(1)
                load_fc1(2)
                run_pass(4, [12, 13])
                run_pass(5, [14, 15])
                run_pass(6, [])

            # ---------------- phase 2: FC head ----------------
            with (
                tc.tile_pool(name="fcio", bufs=12) as fcio_pool,
                tc.tile_pool(name="fcps", bufs=4, space="PSUM") as fcps,
                tc.tile_pool(name="fc3ps", bufs=1, space="PSUM") as fc3ps,
            ):
                h3 = []
                for m in range(8):
                    wt = fc1w[m]
                    ps = fcps.tile([128, 512], f32, tag="fc",
                                   name=f"fc1ps_{m}")
                    for k in range(25):
                        nc.tensor.matmul(ps[:],
                                         wt[:, 128 * k:128 * k + 128],
                                         h2[k][:],
                                         start=(k == 0), stop=(k == 24))
                    ot = fcio_pool.tile([128, 512], f16, tag="h3",
                                        name=f"h3_{m}", bufs=8)
                    relu_evac(ot[:], ps[:])
                    h3.append(ot)
                    if m < 5:
                        load_fc1(m + 3)

                # FC3 accumulates k-major into one [10, 512] PSUM chain
                # (out = fc3.T slice as lhsT, h4[k] moving), interleaved
                # into the FC2 chain loop; output is y [10, 512], the
                # host transposes back to [512, 10].
                h4 = []
                ps3 = fc3ps.tile([128, 512], f32, tag="fc3", name="fc3ps")

                for m in range(4):
                    ps = fcps.tile([128, 512], f32, tag="fc",
                                   name=f"fc2ps_{m}")
                    for k in range(8):
                        nc.tensor.matmul(
                            ps[:],
                            fc2w[:, 1024 * m + 128 * k:
                                 1024 * m + 128 * k + 128],
                            h3[k][:],
                            start=(k == 0), stop=(k == 7))
                    ot = fcio_pool.tile([128, 512], f16, tag="h4",
                                        name=f"h4_{m}", bufs=4)
                    if m == 3:
                        # same-bank ACT+DVE splits serialize (v4 trace)
                        # -- single ACT op is the fastest evac here.
                        nc.scalar.activation(ot[:], ps[:], RELU)
                    else:
                        relu_evac(ot[:], ps[:])
                    h4.append(ot)
                    if m >= 1:
                        nc.tensor.matmul(
                            ps3[0:10, :], fc3w[:, 10 * (m - 1):10 * m],
                            h4[m - 1][:],
                            start=(m == 1), stop=False)
                nc.tensor.matmul(ps3[0:10, :], fc3w[:, 30:40], h4[3][:],
                                 start=False, stop=True)

                yt = fcio_pool.tile([128, 512], f32, tag="yt", name="yt",
                                    bufs=1)
                nc.vector.tensor_copy(yt[0:10, :], ps3[0:10, :])
                nc.sync.dma_start(out=y.ap()[:], in_=yt[0:10, :])
    nc.compile()
    return nc


def kernel(x, conv1w, conv2w, fc1, fc2, fc3):
    global LAST_EXEC_NS
    from concourse.bass_utils import run_bass_kernel_spmd

    x = np.ascontiguousarray(np.asarray(x, dtype=np.float32))
    conv1w = np.ascontiguousarray(np.asarray(conv1w, dtype=np.float32))
    conv2w = np.ascontiguousarray(np.asarray(conv2w, dtype=np.float32))
    fc1 = np.ascontiguousarray(np.asarray(fc1, dtype=np.float32))
    fc2 = np.ascontiguousarray(np.asarray(fc2, dtype=np.float32))
    fc3 = np.ascontiguousarray(np.asarray(fc3, dtype=np.float32))

    if not _NC_CACHE:
        _NC_CACHE.append(_build_nc())
    nc = _NC_CACHE[0]

    xpp = _prep_x(x.astype(np.float16))
    w1abc = _prep_w1(conv1w)
    shared = {
        "w1a": w1abc[0], "w1b": w1abc[1], "w1c": w1abc[2],
        "w2t": _prep_w2(conv2w),
        "fc1m": _prep_fc1(fc1),
        "fc2t": _prep_fc2(fc2),
        "fc3t": _prep_fc3(fc3),
    }
    in_maps = [{**shared, "x_pp": xpp[c]} for c in range(N_CORES)]
    res = run_bass_kernel_spmd(nc, in_maps, list(range(N_CORES)))
    LAST_EXEC_NS = res.exec_time_ns
    # y is [10, 512] per core -> [512, 10]
    outs = [np.ascontiguousarray(r["y"].T) for r in res.results]
    return np.ascontiguousarray(np.concatenate(outs, axis=0))


# revision 24
# speedup vs baseline: 1.0843x; 1.0843x over previous
"""TRN2 Bass kernel for nn_CIFAR10_Type1_Template_Unroll (dense_cnn).

Network (per reference): two locally-connected conv layers + 3-layer FC
head, B=4096. Strategy: pure data parallel over 8 NeuronCores (512 batch
each), activations kept on-chip in [feature, batch] layout, batch N=512
on the matmul free dim throughout.

v5 design notes (v3 measured 146.4us; v4's two-queue split halved the
x stream via per-packet round-robin, starved the PE, re-throttled HAM
to 1.2GHz for 14us -> 148us. v5 reverts to ONE queue):
- Everything fp16 (inputs, weights, activations; PSUM accumulate fp32).
  Measured end-to-end error ~1e-3 vs the 2e-2 gate. Halves DMA bytes.
- ALL weight/input DMAs ride the single sync HWDGE queue in EXACT
  first-need order: x r0, w1(rows 0-3), x r1-r3, w2 pass-0 pairs, x r4,
  w1(4-9), ... , w2 pass6, fc2, fc3, fc1 m0..m7, y-out. A single FIFO
  queue gives the head transfer the full ~390 GB/s; queue position IS
  the pacing policy (pool-buffer WAR stalls at the queue head only ever
  delay strictly-later-needed transfers). No SWDGE, no gate tricks.
- x streams as 16 single-row [128,1024] tiles (256KB completion
  granularity) so row 0 lands ~10us; xp pool bufs=10 so x WAR stalls
  can't block the early w2/w1 transfers behind them.
- PE warmup: 6 full-array K=128 matmuls on a memset tile (~2.6us at
  the cold 1.2GHz clock) + 1 filler after each of rows 0-2; the HAM
  un-throttle window then lands during row 0-2 work. Any PE gap >~3.4us
  re-throttles the clock (v4's failure) -- the schedule keeps gaps
  under ~1.5us.
- L1 PSUM evac was v3's hidden bottleneck: one-engine [128,1024] evacs
  (~1.1us) gated the 2-buffer PSUM recycle, stretching L1 from 6.8us
  ideal to ~18us measured. v5 splits every L1 evac across ACT (cols
  0:512, bank A) + DVE (cols 512:1024, bank B) -- legal, different
  banks, verified overlapping in the v4 trace -- and deepens l1ps to
  bufs=3 (6 banks; l2ps gets 2).
- L2 emitted at full-chain granularity (8 k-steps, one pair of
  positions on PE col halves 0-63/64-127), interleaved with the L1
  half-rows each pass unlocks: pass h = [chain, half-row, chain, ...].
- FC head unchanged in math; fcps bufs=4 + fc1 weights prefetched from
  the sync queue (bufs=3, WAR self-paced) remove the inter-chain
  stalls v3/v4 showed. FC3 interleaves into the FC2 chain loop.
"""
import sys

if '/opt/trn_rl_repo' not in sys.path:
    sys.path.insert(0, '/opt/trn_rl_repo')

import numpy as np

N_CORES = 8
BS = 512
WARM_N = 11
LAST_EXEC_NS = None

# ----------------------------------------------------------------- host prep

def _prep_x(x):
    """x [B,3,32,32] -> [N_CORES, 16, 128, 1024] f16 row tiles.

    part = 32*i + 16*q + f; pair p=4g+i covers w1 in {2p,2p+1}; q = w1
    parity; f = c*4 + kh*2 + kw (12..15 zero-pad). Free dim = (g, batch).
    """
    ncr = x.shape[0] // BS
    xr = x.reshape(ncr, BS, 3, 16, 2, 2, 4, 2, 2)   # s,b,c,r,kh,g,i,q,kw
    xt = xr.transpose(0, 3, 5, 6, 7, 2, 4, 8, 1)    # s,r,g,i,q,c,kh,kw,b
    xt = xt.reshape(ncr, 16, 2, 4, 2, 12, BS)
    xpp = np.zeros((ncr, 16, 2, 4, 2, 16, BS), np.float16)
    xpp[..., :12, :] = xt
    # -> s, r, (i,q,f)=128, (g,b)=1024
    xpp = xpp.reshape(ncr, 16, 2, 128, BS).transpose(0, 1, 3, 2, 4)
    return np.ascontiguousarray(xpp.reshape(ncr, 16, 128, 1024))


def _prep_w1(conv1w):
    """conv1w [64,256,3,2,2] -> [128, 16*256] f16 block-diag strips.

    [p, r*256 + g*128 + c]: strip part p = 32i+16qp+f holds, for parity
    qp, features f -> out channel block c = 64*q + o with q==qp.
    """
    w1r = conv1w.reshape(64, 16, 16, 3, 2, 2)
    wt = w1r.transpose(1, 2, 3, 4, 5, 0).reshape(16, 16, 12, 64)
    wtp = np.zeros((16, 16, 16, 64), np.float32)
    wtp[:, :, :12, :] = wt
    wtp = wtp.reshape(16, 2, 4, 2, 16, 64)          # r,g,i,qp,f,o
    w1t = np.zeros((16, 2, 4, 2, 16, 2, 64), np.float32)
    w1t[:, :, :, 0, :, 0, :] = wtp[:, :, :, 0, :, :]
    w1t[:, :, :, 1, :, 1, :] = wtp[:, :, :, 1, :, :]
    w1t = w1t.reshape(16, 2, 128, 128)              # r,g,p,c
    w1t = w1t.transpose(2, 0, 1, 3)                 # p,r,g,c
    full = w1t.reshape(128, 16 * 256).astype(np.float16)
    # three DRAM-contiguous chunks in need order (rows 0-3 / 4-9 / 10-15):
    # a strided [:, lo:hi] slice of one [128,4096] tensor drains at
    # ~75-100 GB/s (2KB pieces at 8KB stride) and stalls the FIFO queue.
    return (np.ascontiguousarray(full[:, 0:1024]),
            np.ascontiguousarray(full[:, 1024:2560]),
            np.ascontiguousarray(full[:, 2560:4096]))


def _h2_posmap():
    pm = np.full((25, 2), -1, np.int64)
    for T in range(21):
        rr, j = divmod(T, 3)
        pm[T, 0] = rr * 7 + 2 * j
        pm[T, 1] = rr * 7 + 2 * j + 1
    for pi in range(4):
        r0, r1 = 2 * pi, 2 * pi + 1
        pm[21 + pi, 0] = r0 * 7 + 6
        if r1 < 7:
            pm[21 + pi, 1] = r1 * 7 + 6
    return pm


# pair-tile consumption order: pass h emits pairs [3h, 3h+1, 3h+2] plus
# cross pairs 21/22/23+24 at passes 2/4/6; w2 DRAM tiles are stored in
# this exact order so each pass is one contiguous DMA.
_W2_ORDER = [0, 1, 2, 3, 4, 5, 6, 7, 8, 21, 9, 10, 11, 12, 13, 14, 22,
             15, 16, 17, 18, 19, 20, 23, 24]
_W2_SLOT = {T: s for s, T in enumerate(_W2_ORDER)}


def _prep_w2(conv2w):
    """conv2w [64,49,64,4,4] -> [25, 128, 1024] f16 pair tiles in
    consumption (_W2_ORDER) order.

    Per position: [128=(q,c), 512=(kh,t,o)]; pair tile free dim =
    (member u, 512).
    """
    w2r = conv2w.reshape(64, 7, 7, 64, 4, 4)
    v = w2r.transpose(1, 2, 3, 4, 5, 0)             # h,w,c,kh,kw,o
    v = v.reshape(7, 7, 64, 4, 2, 2, 64)            # h,w,c,kh,t,q,o
    v = v.transpose(0, 1, 5, 2, 3, 4, 6)            # h,w,q,c,kh,t,o
    pos = v.reshape(49, 128, 512)
    pm = _h2_posmap()
    out = np.zeros((25, 128, 1024), np.float16)
    for T in range(25):
        s = _W2_SLOT[T]
        out[s, :, 0:512] = pos[pm[T, 0]]
        if pm[T, 1] >= 0:
            out[s, :, 512:1024] = pos[pm[T, 1]]
    return np.ascontiguousarray(out)


def _prep_fc1(fc1):
    """fc1 [1024, 3136] -> [8, 128, 3200] f16, k in h2-tile (T) order."""
    pm = _h2_posmap()
    fc1p = fc1.reshape(1024, 64, 49)
    fc1hat = np.zeros((1024, 25, 2, 64), np.float32)
    for T in range(25):
        for u in range(2):
            p = pm[T, u]
            if p >= 0:
                fc1hat[:, T, u, :] = fc1p[:, :, p]
    a = fc1hat.reshape(8, 128, 25, 128).transpose(0, 3, 2, 1)   # m,kp,k,mc
    return np.ascontiguousarray(a.reshape(8, 128, 3200)).astype(np.float16)


def _prep_fc2(fc2):
    """fc2 [512, 1024] -> [128, 4096] f16: [kp, (m k mc)]."""
    a = fc2.reshape(4, 128, 8, 128)                 # m,mc,k,kp
    a = a.transpose(3, 0, 2, 1)                     # kp,m,k,mc
    return np.ascontiguousarray(a.reshape(128, 4096)).astype(np.float16)


def _prep_fc3(fc3):
    """fc3 [10, 512] -> [128, 40] f16: [kp, (k o)]."""
    a = fc3.T.reshape(4, 128, 10)                   # k,kp,o
    a = a.transpose(1, 0, 2)                        # kp,k,o
    return np.ascontiguousarray(a.reshape(128, 40)).astype(np.float16)


# --------------------------------------------------------------- bass kernel

_NC_CACHE = []


def _build_nc():
    import concourse.bass as bass
    import concourse.mybir as mybir
    from concourse import bacc
    from concourse.tile import TileContext

    f32 = mybir.dt.float32
    f16 = mybir.dt.float16
    RELU = mybir.ActivationFunctionType.Relu

    nc = bacc.Bacc("TRN2", target_bir_lowering=False, debug=False,
                   num_devices=N_CORES)
    x_pp = nc.dram_tensor("x_pp", [16, 128, 1024], f16, kind="ExternalInput")
    w1a = nc.dram_tensor("w1a", [128, 1024], f16, kind="ExternalInput")
    w1b = nc.dram_tensor("w1b", [128, 1536], f16, kind="ExternalInput")
    w1c = nc.dram_tensor("w1c", [128, 1536], f16, kind="ExternalInput")
    w2t = nc.dram_tensor("w2t", [25, 128, 1024], f16, kind="ExternalInput")
    fc1m = nc.dram_tensor("fc1m", [8, 128, 3200], f16, kind="ExternalInput")
    fc2t = nc.dram_tensor("fc2t", [128, 4096], f16, kind="ExternalInput")
    fc3t = nc.dram_tensor("fc3t", [128, 40], f16, kind="ExternalInput")
    y = nc.dram_tensor("y", [10, 512], f32, kind="ExternalOutput")

    pm = _h2_posmap()
    # pass h -> pair tiles, in chain order
    pass_pairs = {h: [3 * h + j for j in range(3)] for h in range(7)}
    pass_pairs[2].append(21)
    pass_pairs[4].append(22)
    pass_pairs[6].extend([23, 24])

    ectr = [0]

    with TileContext(nc) as tc:
        with (
            tc.tile_pool(name="h2pool", bufs=25) as h2pool,
            tc.tile_pool(name="wpool", bufs=4) as wpool,
        ):
            h2 = [h2pool.tile([128, 512], f16, tag="h2", name=f"h2_{T}")
                  for T in range(25)]

            def relu_evac(dst, src):
                if ectr[0] % 2 == 0:
                    nc.scalar.activation(dst, src, RELU)
                else:
                    nc.vector.tensor_scalar_max(dst, src, 0.0)
                ectr[0] += 1

            # The warm-tile memset rides GpSimd: that queue carries
            # nothing else, so the tile is ready the moment the engine
            # preambles finish and the first warm matmul can issue.
            warm = wpool.tile([128, 512], f16, tag="warm", name="warm",
                              bufs=1)
            nc.gpsimd.memset(warm[:], 0.0)

            # fc1 weights stream from the sync queue tail (after all
            # phase-1 transfers); bufs=3 WAR stalls at the queue head
            # self-pace m3..m7 behind the FC1 chains that free their
            # buffers. Nothing later on the queue except y-out.
            fc1w = [None] * 8

            def load_fc1(m):
                wt = wpool.tile([128, 3200], f16, tag="fc1w",
                                name=f"fc1w_{m}", bufs=3)
                nc.sync.dma_start(out=wt[:], in_=fc1m.ap()[m])
                fc1w[m] = wt

            # ---------------- phase 1: L1 + L2 interleaved ----------------
            with (
                tc.tile_pool(name="xp", bufs=10) as xp_pool,
                tc.tile_pool(name="w1p", bufs=1) as w1_pool,
                tc.tile_pool(name="w2p", bufs=3) as w2_pool,
                tc.tile_pool(name="o1p", bufs=32) as o1_pool,
                tc.tile_pool(name="l1ps", bufs=3, space="PSUM") as l1ps,
                tc.tile_pool(name="l2ps", bufs=2, space="PSUM") as l2ps,
            ):
                xt = [None] * 16

                def load_x(r):
                    t = xp_pool.tile([128, 1024], f16, tag="xp",
                                     name=f"xp_{r}")
                    nc.sync.dma_start(out=t[:], in_=x_pp.ap()[r])
                    xt[r] = t

                w1tile = w1_pool.tile([128, 4096], f16, tag="w1",
                                      name="w1")

                _w1chunks = [(w1a, 0, 1024), (w1b, 1024, 2560),
                             (w1c, 2560, 4096)]

                def load_w1(ci):
                    src, lo, hi = _w1chunks[ci]
                    nc.sync.dma_start(out=w1tile[:, lo:hi],
                                      in_=src.ap())

                # w2 tiles: pass 0 as per-pair 256KB DMAs (fine-grain
                # early arrival), passes 1-6 as whole-pass DMAs.
                w2tiles = {}

                def load_w2_pair(T, w2tl, j):
                    s = _W2_SLOT[T]
                    nc.sync.dma_start(
                        out=w2tl[:, 1024 * j:1024 * j + 1024],
                        in_=w2t.ap()[s])
                    w2tiles[T] = w2tl[:, 1024 * j:1024 * j + 1024]

                def alloc_w2(h):
                    n = len(pass_pairs[h])
                    return w2_pool.tile([128, 1024 * n], f16, tag="w2",
                                        name=f"w2p_{h}", bufs=3)

                def load_w2_pass(h):
                    # per-pair 256KB DMAs: each is DRAM-contiguous; a
                    # whole-pass gathered transfer (2KB pieces at 256KB
                    # stride) drains ~4x slower and stalls the queue.
                    t = alloc_w2(h)
                    for j, T in enumerate(pass_pairs[h]):
                        load_w2_pair(T, t, j)

                # ---- sync-queue DMA emission, exact first-need order.
                # Only fresh pool buffers here (10 x tiles, 3 w2 slots,
                # the single w1 tile); recycled buffers are emitted
                # later, after their previous tenant's readers, so the
                # emission-order WAR tracking stays sound.
                load_x(0)
                load_w1(0)                  # rows 0-3
                load_x(1)
                load_x(2)
                w2t0 = alloc_w2(0)
                load_w2_pair(0, w2t0, 0)
                load_x(3)
                load_w2_pair(1, w2t0, 1)
                load_w2_pair(2, w2t0, 2)
                load_x(4)
                load_w1(1)                  # rows 4-9
                load_x(5)
                w2t1 = alloc_w2(1)
                load_w2_pair(3, w2t1, 0)
                load_x(6)
                load_w2_pair(4, w2t1, 1)
                load_x(7)
                load_w2_pair(5, w2t1, 2)
                load_x(8)
                load_w1(2)                  # rows 10-15
                load_x(9)
                load_w2_pass(2)

                # PE warmup: full-array (K=128, M=128) matmuls on the
                # memset tile so HAM un-throttles during the DMA ramp.
                wps = l2ps.tile([128, 512], f32, tag="l2", name="warm_ps")

                def emit_warm(n):
                    for _ in range(n):
                        nc.tensor.matmul(wps[:], warm[:, 0:128], warm[:],
                                         start=True, stop=True)

                out1 = [[None] * 8 for _ in range(16)]

                def emit_l1_half_unpinned(r, g):
                    # 2 half-groups of 2 concurrent K=32 strip matmuls
                    # (tile_position row groups); each (g, half) lands
                    # in a [128,1024] PSUM tile whose evac is split
                    # ACT (cols 0:512, bank A) / DVE (512:1024, bank B)
                    # to halve the recycle latency.
                    w1row = w1tile[:, 256 * r:256 * r + 256]
                    for half in range(2):
                        ps = l1ps.tile([128, 1024], f32, tag="l1",
                                       name=f"l1ps_{r}_{g}_{half}")
                        for sub in range(2):
                            i = half + 2 * sub
                            nc.tensor.matmul(
                                ps[:, 512 * sub:512 * sub + 512],
                                w1row[32 * i:32 * i + 32,
                                      128 * g:128 * g + 128],
                                xt[r][32 * i:32 * i + 32,
                                      512 * g:512 * g + 512],
                                start=True, stop=True,
                                tile_position=(32 * i, 0))
                        ot = o1_pool.tile([128, 1024], f16, tag="o1",
                                          name=f"o1_{r}_{g}_{half}")
                        nc.scalar.activation(ot[:, 0:512],
                                             ps[:, 0:512], RELU)
                        nc.vector.tensor_scalar_max(ot[:, 512:1024],
                                                    ps[:, 512:1024],
                                                    0.0)
                        for sub in range(2):
                            out1[r][4 * g + half + 2 * sub] = \
                                ot[:, 512 * sub:512 * sub + 512]

                def emit_l1_half(r, g):
                    with tick(0.00045):
                        emit_l1_half_unpinned(r, g)

                def emit_l1_row(r):
                    emit_l1_half(r, 0)
                    emit_l1_half(r, 1)

                def emit_chain_part(T, k0, k1, ps):
                    # One position pair: A/B chains on PE col strips
                    # 0-63 / 64-127 share one [128,512] PSUM tile split
                    # by partition range; one evac covers the pair.
                    pA, pB = pm[T]
                    hA, wA = divmod(int(pA), 7)
                    hB, wB = (None, None) if pB < 0 else divmod(int(pB), 7)
                    wt2 = w2tiles[T]
                    if ps is None:
                        ps = l2ps.tile([128, 512], f32, tag="l2",
                                       name=f"l2ps_{T}")
                    for kt in range(k0, k1):
                        kh, t = divmod(kt, 2)
                        nc.tensor.matmul(
                            ps[0:64, :],
                            wt2[:, 64 * kt:64 * kt + 64],
                            out1[2 * hA + kh][wA + t],
                            start=(kt == 0), stop=(kt == 7),
                            tile_position=(0, 0))
                        if hB is not None:
                            nc.tensor.matmul(
                                ps[64:128, :],
                                wt2[:, 512 + 64 * kt:512 + 64 * kt + 64],
                                out1[2 * hB + kh][wB + t],
                                start=(kt == 0), stop=(kt == 7),
                                tile_position=(0, 64))
                    if k1 == 8:
                        relu_evac(h2[T][:], ps[:])
                    return ps

                def emit_chain_part_pinned(T, k0, k1, ps):
                    with tick(0.0019 * (k1 - k0) / 8):
                        return emit_chain_part(T, k0, k1, ps)

                def emit_chain(T):
                    emit_chain_part_pinned(T, 0, 8, None)

                def run_pass(h, rows):
                    # interleave at half-row granularity: a chain (or
                    # half-chain) between any two L1 half-rows gives the
                    # l1ps recycle (gated by the laggy DVE evac queue)
                    # enough slack. When a pass has more half-rows than
                    # chains, SPLIT trailing chains into two 4-step
                    # chunks -- two back-to-back half-rows stall ~1-2us
                    # on the PSUM WAR (v6 trace), a chunk boundary only
                    # ~0.1us.
                    ts = pass_pairs[h]
                    halves = [(r, g) for r in rows for g in range(2)]
                    n_extra = max(0, len(halves) - len(ts))
                    hi = 0

                    def next_half():
                        nonlocal hi
                        if hi < len(halves):
                            emit_l1_half(*halves[hi])
                            hi += 1

                    for j, T in enumerate(ts):
                        if j >= len(ts) - n_extra:
                            ps = emit_chain_part_pinned(T, 0, 4, None)
                            next_half()
                            emit_chain_part_pinned(T, 4, 8, ps)
                        else:
                            emit_chain(T)
                        next_half()

                # ---- PE emission: warmups + rows 0-3, then passes.
                # Recycled-buffer DMAs are emitted at the first point
                # after their WAR predecessor's readers; their queue
                # position still honors first-need order.
                #
                # Phase-1 PE units are PINNED to a paced logical clock
                # (tile_wait_until gates only the Tile scheduler's sim,
                # not runtime): without pins the scheduler weaves L1
                # half-rows INTO chain-round gaps and even splits the
                # A/B col-pair of one k-step (v7 trace, 15-21us: ~320ns
                # rounds from per-switch weight-reload exposure).
                # NOTE: forcing strict unit order via start-floor pins
                # measured 6us SLOWER (v9) -- the scheduler's fine
                # interleave works around genuinely-late dependencies
                # (DVE evac halves, DMA arrivals) better than a rigid
                # order. Keep pins as inert end-floors (no-ops in
                # practice; v8 == v7 within noise).
                pin = [0.004]

                def tick(d):
                    pin[0] += d
                    return tc.tile_wait_until(pin[0])

                emit_warm(WARM_N)
                emit_l1_row(0)
                load_x(10)
                emit_warm(2)
                emit_l1_row(1)
                load_x(11)
                emit_warm(2)
                emit_l1_row(2)
                load_x(12)
                emit_warm(1)
                emit_l1_row(3)
                load_x(13)
                emit_warm(1)
                run_pass(0, [4, 5])
                load_x(14)
                load_x(15)
                load_w2_pass(3)
                run_pass(1, [6, 7])
                load_w2_pass(4)
                run_pass(2, [8, 9])
                load_w2_pass(5)
                run_pass(3, [10, 11])
                load_w2_pass(6)
                fc2w = wpool.tile([128, 4096], f16, tag="fc2w",
                                  name="fc2w", bufs=1)
                nc.sync.dma_start(out=fc2w[:], in_=fc2t.ap())
                fc3w = wpool.tile([128, 40], f16, tag="fc3w",
                                  name="fc3w", bufs=1)
                nc.sync.dma_start(out=fc3w[:], in_=fc3t.ap())
                load_fc1(0)
                load_fc1(1)
                load_fc1(2)
                run_pass(4, [12, 13])
                run_pass(5, [14, 15])
                run_pass(6, [])

            # ---------------- phase 2: FC head ----------------
            with (
                tc.tile_pool(name="fcio", bufs=12) as fcio_pool,
                tc.tile_pool(name="fcps", bufs=4, space="PSUM") as fcps,
                tc.tile_pool(name="fc3ps", bufs=1, space="PSUM") as fc3ps,
            ):
                h3 = []
                for m in range(8):
                    wt = fc1w[m]
                    ps = fcps.tile([128, 512], f32, tag="fc",
                                   name=f"fc1ps_{m}")
                    for k in range(25):
                        nc.tensor.matmul(ps[:],
                                         wt[:, 128 * k:128 * k + 128],
                                         h2[k][:],
                                         start=(k == 0), stop=(k == 24))
                    ot = fcio_pool.tile([128, 512], f16, tag="h3",
                                        name=f"h3_{m}", bufs=8)
                    relu_evac(ot[:], ps[:])
                    h3.append(ot)
                    if m < 5:
                        load_fc1(m + 3)

                # FC3 accumulates k-major into one [10, 512] PSUM chain
                # (out = fc3.T slice as lhsT, h4[k] moving), interleaved
                # into the FC2 chain loop; output is y [10, 512], the
                # host transposes back to [512, 10].
                h4 = []
                ps3 = fc3ps.tile([128, 512], f32, tag="fc3", name="fc3ps")

                for m in range(4):
                    ps = fcps.tile([128, 512], f32, tag="fc",
                                   name=f"fc2ps_{m}")
                    for k in range(8):
                        nc.tensor.matmul(
                            ps[:],
                            fc2w[:, 1024 * m + 128 * k:
                                 1024 * m + 128 * k + 128],
                            h3[k][:],
                            start=(k == 0), stop=(k == 7))
                    ot = fcio_pool.tile([128, 512], f16, tag="h4",
                                        name=f"h4_{m}", bufs=4)
                    if m == 3:
                        # same-bank ACT+DVE splits serialize (v4 trace)
                        # -- single ACT op is the fastest evac here.
                        nc.scalar.activation(ot[:], ps[:], RELU)
                    else:
                        relu_evac(ot[:], ps[:])
                    h4.append(ot)
                    if m >= 1:
                        nc.tensor.matmul(
                            ps3[0:10, :], fc3w[:, 10 * (m - 1):10 * m],
                            h4[m - 1][:],
                            start=(m == 1), stop=False)
                nc.tensor.matmul(ps3[0:10, :], fc3w[:, 30:40], h4[3][:],
                                 start=False, stop=True)

                yt = fcio_pool.tile([128, 512], f32, tag="yt", name="yt",
                                    bufs=1)
                nc.vector.tensor_copy(yt[0:10, :], ps3[0:10, :])
                nc.sync.dma_start(out=y.ap()[:], in_=yt[0:10, :])
    nc.compile()
    return nc


def kernel(x, conv1w, conv2w, fc1, fc2, fc3):
    global LAST_EXEC_NS
    from concourse.bass_utils import run_bass_kernel_spmd

    x = np.ascontiguousarray(np.asarray(x, dtype=np.float32))
    conv1w = np.ascontiguousarray(np.asarray(conv1w, dtype=np.float32))
    conv2w = np.ascontiguousarray(np.asarray(conv2w, dtype=np.float32))
    fc1 = np.ascontiguousarray(np.asarray(fc1, dtype=np.float32))
    fc2 = np.ascontiguousarray(np.asarray(fc2, dtype=np.float32))
    fc3 = np.ascontiguousarray(np.asarray(fc3, dtype=np.float32))

    if not _NC_CACHE:
        _NC_CACHE.append(_build_nc())
    nc = _NC_CACHE[0]

    xpp = _prep_x(x.astype(np.float16))
    w1abc = _prep_w1(conv1w)
    shared = {
        "w1a": w1abc[0], "w1b": w1abc[1], "w1c": w1abc[2],
        "w2t": _prep_w2(conv2w),
        "fc1m": _prep_fc1(fc1),
        "fc2t": _prep_fc2(fc2),
        "fc3t": _prep_fc3(fc3),
    }
    in_maps = [{**shared, "x_pp": xpp[c]} for c in range(N_CORES)]
    res = run_bass_kernel_spmd(nc, in_maps, list(range(N_CORES)))
    LAST_EXEC_NS = res.exec_time_ns
    # y is [10, 512] per core -> [512, 10]
    outs = [np.ascontiguousarray(r["y"].T) for r in res.results]
    return np.ascontiguousarray(np.concatenate(outs, axis=0))


# revision 25
# speedup vs baseline: 1.0884x; 1.0038x over previous
"""TRN2 Bass kernel for nn_CIFAR10_Type1_Template_Unroll (dense_cnn).

Network (per reference): two locally-connected conv layers + 3-layer FC
head, B=4096. Strategy: pure data parallel over 8 NeuronCores (512 batch
each), activations kept on-chip in [feature, batch] layout, batch N=512
on the matmul free dim throughout.

v5 design notes (v3 measured 146.4us; v4's two-queue split halved the
x stream via per-packet round-robin, starved the PE, re-throttled HAM
to 1.2GHz for 14us -> 148us. v5 reverts to ONE queue):
- Everything fp16 (inputs, weights, activations; PSUM accumulate fp32).
  Measured end-to-end error ~1e-3 vs the 2e-2 gate. Halves DMA bytes.
- ALL weight/input DMAs ride the single sync HWDGE queue in EXACT
  first-need order: x r0, w1(rows 0-3), x r1-r3, w2 pass-0 pairs, x r4,
  w1(4-9), ... , w2 pass6, fc2, fc3, fc1 m0..m7, y-out. A single FIFO
  queue gives the head transfer the full ~390 GB/s; queue position IS
  the pacing policy (pool-buffer WAR stalls at the queue head only ever
  delay strictly-later-needed transfers). No SWDGE, no gate tricks.
- x streams as 16 single-row [128,1024] tiles (256KB completion
  granularity) so row 0 lands ~10us; xp pool bufs=10 so x WAR stalls
  can't block the early w2/w1 transfers behind them.
- PE warmup: 6 full-array K=128 matmuls on a memset tile (~2.6us at
  the cold 1.2GHz clock) + 1 filler after each of rows 0-2; the HAM
  un-throttle window then lands during row 0-2 work. Any PE gap >~3.4us
  re-throttles the clock (v4's failure) -- the schedule keeps gaps
  under ~1.5us.
- L1 PSUM evac was v3's hidden bottleneck: one-engine [128,1024] evacs
  (~1.1us) gated the 2-buffer PSUM recycle, stretching L1 from 6.8us
  ideal to ~18us measured. v5 splits every L1 evac across ACT (cols
  0:512, bank A) + DVE (cols 512:1024, bank B) -- legal, different
  banks, verified overlapping in the v4 trace -- and deepens l1ps to
  bufs=3 (6 banks; l2ps gets 2).
- L2 emitted at full-chain granularity (8 k-steps, one pair of
  positions on PE col halves 0-63/64-127), interleaved with the L1
  half-rows each pass unlocks: pass h = [chain, half-row, chain, ...].
- FC head unchanged in math; fcps bufs=4 + fc1 weights prefetched from
  the sync queue (bufs=3, WAR self-paced) remove the inter-chain
  stalls v3/v4 showed. FC3 interleaves into the FC2 chain loop.
"""
import sys

if '/opt/trn_rl_repo' not in sys.path:
    sys.path.insert(0, '/opt/trn_rl_repo')

import numpy as np

N_CORES = 8
BS = 512
WARM_N = 12
LAST_EXEC_NS = None

# ----------------------------------------------------------------- host prep

def _prep_x(x):
    """x [B,3,32,32] -> [N_CORES, 16, 128, 1024] f16 row tiles.

    part = 32*i + 16*q + f; pair p=4g+i covers w1 in {2p,2p+1}; q = w1
    parity; f = c*4 + kh*2 + kw (12..15 zero-pad). Free dim = (g, batch).
    """
    ncr = x.shape[0] // BS
    xr = x.reshape(ncr, BS, 3, 16, 2, 2, 4, 2, 2)   # s,b,c,r,kh,g,i,q,kw
    xt = xr.transpose(0, 3, 5, 6, 7, 2, 4, 8, 1)    # s,r,g,i,q,c,kh,kw,b
    xt = xt.reshape(ncr, 16, 2, 4, 2, 12, BS)
    xpp = np.zeros((ncr, 16, 2, 4, 2, 16, BS), np.float16)
    xpp[..., :12, :] = xt
    # -> s, r, (i,q,f)=128, (g,b)=1024
    xpp = xpp.reshape(ncr, 16, 2, 128, BS).transpose(0, 1, 3, 2, 4)
    return np.ascontiguousarray(xpp.reshape(ncr, 16, 128, 1024))


def _prep_w1(conv1w):
    """conv1w [64,256,3,2,2] -> [128, 16*256] f16 block-diag strips.

    [p, r*256 + g*128 + c]: strip part p = 32i+16qp+f holds, for parity
    qp, features f -> out channel block c = 64*q + o with q==qp.
    """
    w1r = conv1w.reshape(64, 16, 16, 3, 2, 2)
    wt = w1r.transpose(1, 2, 3, 4, 5, 0).reshape(16, 16, 12, 64)
    wtp = np.zeros((16, 16, 16, 64), np.float32)
    wtp[:, :, :12, :] = wt
    wtp = wtp.reshape(16, 2, 4, 2, 16, 64)          # r,g,i,qp,f,o
    w1t = np.zeros((16, 2, 4, 2, 16, 2, 64), np.float32)
    w1t[:, :, :, 0, :, 0, :] = wtp[:, :, :, 0, :, :]
    w1t[:, :, :, 1, :, 1, :] = wtp[:, :, :, 1, :, :]
    w1t = w1t.reshape(16, 2, 128, 128)              # r,g,p,c
    w1t = w1t.transpose(2, 0, 1, 3)                 # p,r,g,c
    full = w1t.reshape(128, 16 * 256).astype(np.float16)
    # three DRAM-contiguous chunks in need order (rows 0-3 / 4-9 / 10-15):
    # a strided [:, lo:hi] slice of one [128,4096] tensor drains at
    # ~75-100 GB/s (2KB pieces at 8KB stride) and stalls the FIFO queue.
    return (np.ascontiguousarray(full[:, 0:1024]),
            np.ascontiguousarray(full[:, 1024:2560]),
            np.ascontiguousarray(full[:, 2560:4096]))


def _h2_posmap():
    pm = np.full((25, 2), -1, np.int64)
    for T in range(21):
        rr, j = divmod(T, 3)
        pm[T, 0] = rr * 7 + 2 * j
        pm[T, 1] = rr * 7 + 2 * j + 1
    for pi in range(4):
        r0, r1 = 2 * pi, 2 * pi + 1
        pm[21 + pi, 0] = r0 * 7 + 6
        if r1 < 7:
            pm[21 + pi, 1] = r1 * 7 + 6
    return pm


# pair-tile consumption order: pass h emits pairs [3h, 3h+1, 3h+2] plus
# cross pairs 21/22/23+24 at passes 2/4/6; w2 DRAM tiles are stored in
# this exact order so each pass is one contiguous DMA.
_W2_ORDER = [0, 1, 2, 3, 4, 5, 6, 7, 8, 21, 9, 10, 11, 12, 13, 14, 22,
             15, 16, 17, 18, 19, 20, 23, 24]
_W2_SLOT = {T: s for s, T in enumerate(_W2_ORDER)}


def _prep_w2(conv2w):
    """conv2w [64,49,64,4,4] -> [25, 128, 1024] f16 pair tiles in
    consumption (_W2_ORDER) order.

    Per position: [128=(q,c), 512=(kh,t,o)]; pair tile free dim =
    (member u, 512).
    """
    w2r = conv2w.reshape(64, 7, 7, 64, 4, 4)
    v = w2r.transpose(1, 2, 3, 4, 5, 0)             # h,w,c,kh,kw,o
    v = v.reshape(7, 7, 64, 4, 2, 2, 64)            # h,w,c,kh,t,q,o
    v = v.transpose(0, 1, 5, 2, 3, 4, 6)            # h,w,q,c,kh,t,o
    pos = v.reshape(49, 128, 512)
    pm = _h2_posmap()
    out = np.zeros((25, 128, 1024), np.float16)
    for T in range(25):
        s = _W2_SLOT[T]
        out[s, :, 0:512] = pos[pm[T, 0]]
        if pm[T, 1] >= 0:
            out[s, :, 512:1024] = pos[pm[T, 1]]
    return np.ascontiguousarray(out)


def _prep_fc1(fc1):
    """fc1 [1024, 3136] -> [8, 128, 3200] f16, k in h2-tile (T) order."""
    pm = _h2_posmap()
    fc1p = fc1.reshape(1024, 64, 49)
    fc1hat = np.zeros((1024, 25, 2, 64), np.float32)
    for T in range(25):
        for u in range(2):
            p = pm[T, u]
            if p >= 0:
                fc1hat[:, T, u, :] = fc1p[:, :, p]
    a = fc1hat.reshape(8, 128, 25, 128).transpose(0, 3, 2, 1)   # m,kp,k,mc
    return np.ascontiguousarray(a.reshape(8, 128, 3200)).astype(np.float16)


def _prep_fc2(fc2):
    """fc2 [512, 1024] -> [128, 4096] f16: [kp, (m k mc)]."""
    a = fc2.reshape(4, 128, 8, 128)                 # m,mc,k,kp
    a = a.transpose(3, 0, 2, 1)                     # kp,m,k,mc
    return np.ascontiguousarray(a.reshape(128, 4096)).astype(np.float16)


def _prep_fc3(fc3):
    """fc3 [10, 512] -> [128, 40] f16: [kp, (k o)]."""
    a = fc3.T.reshape(4, 128, 10)                   # k,kp,o
    a = a.transpose(1, 0, 2)                        # kp,k,o
    return np.ascontiguousarray(a.reshape(128, 40)).astype(np.float16)


# --------------------------------------------------------------- bass kernel

_NC_CACHE = []


def _build_nc():
    import concourse.bass as bass
    import concourse.mybir as mybir
    from concourse import bacc
    from concourse.tile import TileContext

    f32 = mybir.dt.float32
    f16 = mybir.dt.float16
    RELU = mybir.ActivationFunctionType.Relu

    nc = bacc.Bacc("TRN2", target_bir_lowering=False, debug=False,
                   num_devices=N_CORES)
    x_pp = nc.dram_tensor("x_pp", [16, 128, 1024], f16, kind="ExternalInput")
    w1a = nc.dram_tensor("w1a", [128, 1024], f16, kind="ExternalInput")
    w1b = nc.dram_tensor("w1b", [128, 1536], f16, kind="ExternalInput")
    w1c = nc.dram_tensor("w1c", [128, 1536], f16, kind="ExternalInput")
    w2t = nc.dram_tensor("w2t", [25, 128, 1024], f16, kind="ExternalInput")
    fc1m = nc.dram_tensor("fc1m", [8, 128, 3200], f16, kind="ExternalInput")
    fc2t = nc.dram_tensor("fc2t", [128, 4096], f16, kind="ExternalInput")
    fc3t = nc.dram_tensor("fc3t", [128, 40], f16, kind="ExternalInput")
    y = nc.dram_tensor("y", [10, 512], f32, kind="ExternalOutput")

    pm = _h2_posmap()
    # pass h -> pair tiles, in chain order
    pass_pairs = {h: [3 * h + j for j in range(3)] for h in range(7)}
    pass_pairs[2].append(21)
    pass_pairs[4].append(22)
    pass_pairs[6].extend([23, 24])

    ectr = [0]

    with TileContext(nc) as tc:
        with (
            tc.tile_pool(name="h2pool", bufs=25) as h2pool,
            tc.tile_pool(name="wpool", bufs=4) as wpool,
        ):
            h2 = [h2pool.tile([128, 512], f16, tag="h2", name=f"h2_{T}")
                  for T in range(25)]

            def relu_evac(dst, src):
                if ectr[0] % 2 == 0:
                    nc.scalar.activation(dst, src, RELU)
                else:
                    nc.vector.tensor_scalar_max(dst, src, 0.0)
                ectr[0] += 1

            # The warm-tile memset rides GpSimd: that queue carries
            # nothing else, so the tile is ready the moment the engine
            # preambles finish and the first warm matmul can issue.
            warm = wpool.tile([128, 512], f16, tag="warm", name="warm",
                              bufs=1)
            nc.gpsimd.memset(warm[:], 0.0)

            # fc1 weights stream from the sync queue tail (after all
            # phase-1 transfers); bufs=3 WAR stalls at the queue head
            # self-pace m3..m7 behind the FC1 chains that free their
            # buffers. Nothing later on the queue except y-out.
            fc1w = [None] * 8

            def load_fc1(m):
                wt = wpool.tile([128, 3200], f16, tag="fc1w",
                                name=f"fc1w_{m}", bufs=3)
                nc.sync.dma_start(out=wt[:], in_=fc1m.ap()[m])
                fc1w[m] = wt

            # ---------------- phase 1: L1 + L2 interleaved ----------------
            with (
                tc.tile_pool(name="xp", bufs=10) as xp_pool,
                tc.tile_pool(name="w1p", bufs=1) as w1_pool,
                tc.tile_pool(name="w2p", bufs=3) as w2_pool,
                tc.tile_pool(name="o1p", bufs=32) as o1_pool,
                tc.tile_pool(name="l1ps", bufs=3, space="PSUM") as l1ps,
                tc.tile_pool(name="l2ps", bufs=2, space="PSUM") as l2ps,
            ):
                xt = [None] * 16

                def load_x(r):
                    t = xp_pool.tile([128, 1024], f16, tag="xp",
                                     name=f"xp_{r}")
                    nc.sync.dma_start(out=t[:], in_=x_pp.ap()[r])
                    xt[r] = t

                w1tile = w1_pool.tile([128, 4096], f16, tag="w1",
                                      name="w1")

                _w1chunks = [(w1a, 0, 1024), (w1b, 1024, 2560),
                             (w1c, 2560, 4096)]

                def load_w1(ci):
                    src, lo, hi = _w1chunks[ci]
                    nc.sync.dma_start(out=w1tile[:, lo:hi],
                                      in_=src.ap())

                # w2 tiles: pass 0 as per-pair 256KB DMAs (fine-grain
                # early arrival), passes 1-6 as whole-pass DMAs.
                w2tiles = {}

                def load_w2_pair(T, w2tl, j):
                    s = _W2_SLOT[T]
                    nc.sync.dma_start(
                        out=w2tl[:, 1024 * j:1024 * j + 1024],
                        in_=w2t.ap()[s])
                    w2tiles[T] = w2tl[:, 1024 * j:1024 * j + 1024]

                def alloc_w2(h):
                    n = len(pass_pairs[h])
                    return w2_pool.tile([128, 1024 * n], f16, tag="w2",
                                        name=f"w2p_{h}", bufs=3)

                def load_w2_pass(h):
                    # per-pair 256KB DMAs: each is DRAM-contiguous; a
                    # whole-pass gathered transfer (2KB pieces at 256KB
                    # stride) drains ~4x slower and stalls the queue.
                    t = alloc_w2(h)
                    for j, T in enumerate(pass_pairs[h]):
                        load_w2_pair(T, t, j)

                # ---- sync-queue DMA emission, exact first-need order.
                # Only fresh pool buffers here (10 x tiles, 3 w2 slots,
                # the single w1 tile); recycled buffers are emitted
                # later, after their previous tenant's readers, so the
                # emission-order WAR tracking stays sound.
                load_x(0)
                load_w1(0)                  # rows 0-3
                load_x(1)
                load_x(2)
                w2t0 = alloc_w2(0)
                load_w2_pair(0, w2t0, 0)
                load_x(3)
                load_w2_pair(1, w2t0, 1)
                load_w2_pair(2, w2t0, 2)
                load_x(4)
                load_w1(1)                  # rows 4-9
                load_x(5)
                w2t1 = alloc_w2(1)
                load_w2_pair(3, w2t1, 0)
                load_x(6)
                load_w2_pair(4, w2t1, 1)
                load_x(7)
                load_w2_pair(5, w2t1, 2)
                load_x(8)
                load_w1(2)                  # rows 10-15
                load_x(9)
                load_w2_pass(2)

                # PE warmup: full-array (K=128, M=128) matmuls on the
                # memset tile so HAM un-throttles during the DMA ramp.
                wps = l2ps.tile([128, 512], f32, tag="l2", name="warm_ps")

                def emit_warm(n):
                    for _ in range(n):
                        nc.tensor.matmul(wps[:], warm[:, 0:128], warm[:],
                                         start=True, stop=True)

                out1 = [[None] * 8 for _ in range(16)]

                def emit_l1_half_unpinned(r, g):
                    # 2 half-groups of 2 concurrent K=32 strip matmuls
                    # (tile_position row groups); each (g, half) lands
                    # in a [128,1024] PSUM tile whose evac is split
                    # ACT (cols 0:512, bank A) / DVE (512:1024, bank B)
                    # to halve the recycle latency.
                    w1row = w1tile[:, 256 * r:256 * r + 256]
                    for half in range(2):
                        ps = l1ps.tile([128, 1024], f32, tag="l1",
                                       name=f"l1ps_{r}_{g}_{half}")
                        for sub in range(2):
                            i = half + 2 * sub
                            nc.tensor.matmul(
                                ps[:, 512 * sub:512 * sub + 512],
                                w1row[32 * i:32 * i + 32,
                                      128 * g:128 * g + 128],
                                xt[r][32 * i:32 * i + 32,
                                      512 * g:512 * g + 512],
                                start=True, stop=True,
                                tile_position=(32 * i, 0))
                        ot = o1_pool.tile([128, 1024], f16, tag="o1",
                                          name=f"o1_{r}_{g}_{half}")
                        nc.scalar.activation(ot[:, 0:512],
                                             ps[:, 0:512], RELU)
                        nc.vector.tensor_scalar_max(ot[:, 512:1024],
                                                    ps[:, 512:1024],
                                                    0.0)
                        for sub in range(2):
                            out1[r][4 * g + half + 2 * sub] = \
                                ot[:, 512 * sub:512 * sub + 512]

                def emit_l1_half(r, g):
                    with tick(0.00045):
                        emit_l1_half_unpinned(r, g)

                def emit_l1_row(r):
                    emit_l1_half(r, 0)
                    emit_l1_half(r, 1)

                def emit_chain_part(T, k0, k1, ps):
                    # One position pair: A/B chains on PE col strips
                    # 0-63 / 64-127 share one [128,512] PSUM tile split
                    # by partition range; one evac covers the pair.
                    pA, pB = pm[T]
                    hA, wA = divmod(int(pA), 7)
                    hB, wB = (None, None) if pB < 0 else divmod(int(pB), 7)
                    wt2 = w2tiles[T]
                    if ps is None:
                        ps = l2ps.tile([128, 512], f32, tag="l2",
                                       name=f"l2ps_{T}")
                    for kt in range(k0, k1):
                        kh, t = divmod(kt, 2)
                        nc.tensor.matmul(
                            ps[0:64, :],
                            wt2[:, 64 * kt:64 * kt + 64],
                            out1[2 * hA + kh][wA + t],
                            start=(kt == 0), stop=(kt == 7),
                            tile_position=(0, 0))
                        if hB is not None:
                            nc.tensor.matmul(
                                ps[64:128, :],
                                wt2[:, 512 + 64 * kt:512 + 64 * kt + 64],
                                out1[2 * hB + kh][wB + t],
                                start=(kt == 0), stop=(kt == 7),
                                tile_position=(0, 64))
                    if k1 == 8:
                        relu_evac(h2[T][:], ps[:])
                    return ps

                def emit_chain_part_pinned(T, k0, k1, ps):
                    with tick(0.0019 * (k1 - k0) / 8):
                        return emit_chain_part(T, k0, k1, ps)

                def emit_chain(T):
                    emit_chain_part_pinned(T, 0, 8, None)

                def run_pass(h, rows):
                    # interleave at half-row granularity: a chain (or
                    # half-chain) between any two L1 half-rows gives the
                    # l1ps recycle (gated by the laggy DVE evac queue)
                    # enough slack. When a pass has more half-rows than
                    # chains, SPLIT trailing chains into two 4-step
                    # chunks -- two back-to-back half-rows stall ~1-2us
                    # on the PSUM WAR (v6 trace), a chunk boundary only
                    # ~0.1us.
                    ts = pass_pairs[h]
                    halves = [(r, g) for r in rows for g in range(2)]
                    n_extra = max(0, len(halves) - len(ts))
                    hi = 0

                    def next_half():
                        nonlocal hi
                        if hi < len(halves):
                            emit_l1_half(*halves[hi])
                            hi += 1

                    for j, T in enumerate(ts):
                        if j >= len(ts) - n_extra:
                            ps = emit_chain_part_pinned(T, 0, 4, None)
                            next_half()
                            emit_chain_part_pinned(T, 4, 8, ps)
                        else:
                            emit_chain(T)
                        next_half()

                # ---- PE emission: warmups + rows 0-3, then passes.
                # Recycled-buffer DMAs are emitted at the first point
                # after their WAR predecessor's readers; their queue
                # position still honors first-need order.
                #
                # Phase-1 PE units are PINNED to a paced logical clock
                # (tile_wait_until gates only the Tile scheduler's sim,
                # not runtime): without pins the scheduler weaves L1
                # half-rows INTO chain-round gaps and even splits the
                # A/B col-pair of one k-step (v7 trace, 15-21us: ~320ns
                # rounds from per-switch weight-reload exposure).
                # NOTE: forcing strict unit order via start-floor pins
                # measured 6us SLOWER (v9) -- the scheduler's fine
                # interleave works around genuinely-late dependencies
                # (DVE evac halves, DMA arrivals) better than a rigid
                # order. Keep pins as inert end-floors (no-ops in
                # practice; v8 == v7 within noise).
                pin = [0.004]

                def tick(d):
                    pin[0] += d
                    return tc.tile_wait_until(pin[0])

                emit_warm(WARM_N)
                emit_l1_row(0)
                load_x(10)
                emit_warm(2)
                emit_l1_row(1)
                load_x(11)
                emit_warm(2)
                emit_l1_row(2)
                load_x(12)
                emit_warm(1)
                emit_l1_row(3)
                load_x(13)
                emit_warm(1)
                run_pass(0, [4, 5])
                load_x(14)
                load_x(15)
                load_w2_pass(3)
                run_pass(1, [6, 7])
                load_w2_pass(4)
                run_pass(2, [8, 9])
                load_w2_pass(5)
                run_pass(3, [10, 11])
                load_w2_pass(6)
                fc2w = wpool.tile([128, 4096], f16, tag="fc2w",
                                  name="fc2w", bufs=1)
                nc.sync.dma_start(out=fc2w[:], in_=fc2t.ap())
                fc3w = wpool.tile([128, 40], f16, tag="fc3w",
                                  name="fc3w", bufs=1)
                nc.sync.dma_start(out=fc3w[:], in_=fc3t.ap())
                load_fc1(0)
                load_fc1(1)
                load_fc1(2)
                run_pass(4, [12, 13])
                run_pass(5, [14, 15])
                run_pass(6, [])

            # ---------------- phase 2: FC head ----------------
            with (
                tc.tile_pool(name="fcio", bufs=12) as fcio_pool,
                tc.tile_pool(name="fcps", bufs=4, space="PSUM") as fcps,
                tc.tile_pool(name="fc3ps", bufs=1, space="PSUM") as fc3ps,
            ):
                h3 = []
                for m in range(8):
                    wt = fc1w[m]
                    ps = fcps.tile([128, 512], f32, tag="fc",
                                   name=f"fc1ps_{m}")
                    for k in range(25):
                        nc.tensor.matmul(ps[:],
                                         wt[:, 128 * k:128 * k + 128],
                                         h2[k][:],
                                         start=(k == 0), stop=(k == 24))
                    ot = fcio_pool.tile([128, 512], f16, tag="h3",
                                        name=f"h3_{m}", bufs=8)
                    relu_evac(ot[:], ps[:])
                    h3.append(ot)
                    if m < 5:
                        load_fc1(m + 3)

                # FC3 accumulates k-major into one [10, 512] PSUM chain
                # (out = fc3.T slice as lhsT, h4[k] moving), interleaved
                # into the FC2 chain loop; output is y [10, 512], the
                # host transposes back to [512, 10].
                h4 = []
                ps3 = fc3ps.tile([128, 512], f32, tag="fc3", name="fc3ps")

                for m in range(4):
                    ps = fcps.tile([128, 512], f32, tag="fc",
                                   name=f"fc2ps_{m}")
                    for k in range(8):
                        nc.tensor.matmul(
                            ps[:],
                            fc2w[:, 1024 * m + 128 * k:
                                 1024 * m + 128 * k + 128],
                            h3[k][:],
                            start=(k == 0), stop=(k == 7))
                    ot = fcio_pool.tile([128, 512], f16, tag="h4",
                                        name=f"h4_{m}", bufs=4)
                    if m == 3:
                        # same-bank ACT+DVE splits serialize (v4 trace)
                        # -- single ACT op is the fastest evac here.
                        nc.scalar.activation(ot[:], ps[:], RELU)
                    else:
                        relu_evac(ot[:], ps[:])
                    h4.append(ot)
                    if m >= 1:
                        nc.tensor.matmul(
                            ps3[0:10, :], fc3w[:, 10 * (m - 1):10 * m],
                            h4[m - 1][:],
                            start=(m == 1), stop=False)
                nc.tensor.matmul(ps3[0:10, :], fc3w[:, 30:40], h4[3][:],
                                 start=False, stop=True)

                yt = fcio_pool.tile([128, 512], f32, tag="yt", name="yt",
                                    bufs=1)
                nc.vector.tensor_copy(yt[0:10, :], ps3[0:10, :])
                nc.sync.dma_start(out=y.ap()[:], in_=yt[0:10, :])
    nc.compile()
    return nc


def kernel(x, conv1w, conv2w, fc1, fc2, fc3):
    global LAST_EXEC_NS
    from concourse.bass_utils import run_bass_kernel_spmd

    x = np.ascontiguousarray(np.asarray(x, dtype=np.float32))
    conv1w = np.ascontiguousarray(np.asarray(conv1w, dtype=np.float32))
    conv2w = np.ascontiguousarray(np.asarray(conv2w, dtype=np.float32))
    fc1 = np.ascontiguousarray(np.asarray(fc1, dtype=np.float32))
    fc2 = np.ascontiguousarray(np.asarray(fc2, dtype=np.float32))
    fc3 = np.ascontiguousarray(np.asarray(fc3, dtype=np.float32))

    if not _NC_CACHE:
        _NC_CACHE.append(_build_nc())
    nc = _NC_CACHE[0]

    xpp = _prep_x(x.astype(np.float16))
    w1abc = _prep_w1(conv1w)
    shared = {
        "w1a": w1abc[0], "w1b": w1abc[1], "w1c": w1abc[2],
        "w2t": _prep_w2(conv2w),
        "fc1m": _prep_fc1(fc1),
        "fc2t": _prep_fc2(fc2),
        "fc3t": _prep_fc3(fc3),
    }
    in_maps = [{**shared, "x_pp": xpp[c]} for c in range(N_CORES)]
    res = run_bass_kernel_spmd(nc, in_maps, list(range(N_CORES)))
    LAST_EXEC_NS = res.exec_time_ns
    # y is [10, 512] per core -> [512, 10]
    outs = [np.ascontiguousarray(r["y"].T) for r in res.results]
    return np.ascontiguousarray(np.concatenate(outs, axis=0))


# revision 26
# speedup vs baseline: 1.0886x; 1.0002x over previous
"""TRN2 Bass kernel for nn_CIFAR10_Type1_Template_Unroll (dense_cnn).

Network (per reference): two locally-connected conv layers + 3-layer FC
head, B=4096. Strategy: pure data parallel over 8 NeuronCores (512 batch
each), activations kept on-chip in [feature, batch] layout, batch N=512
on the matmul free dim throughout.

v10 design notes (baseline v3: 146.4us; this version: ~135.7-136.5us
measured, rel err 1.03e-3 vs the 2e-2 gate). Trace-driven findings:
- Everything fp16 (inputs, weights, activations; PSUM accumulate fp32).
- ALL input/weight DMAs ride the single sync HWDGE queue in EXACT
  first-need order: x r0, w1(rows 0-3), x r1-r3, w2 pass-0 pairs, x r4,
  w1(4-9), ... , w2 pass6, fc2, fc3, fc1 m0..m7, y-out. A single FIFO
  queue gives the head transfer the full ~390 GB/s; queue position IS
  the pacing policy. Splitting streams across two HWDGE queues (v4)
  HALVED the critical stream via per-packet round-robin and re-
  throttled the PE clock -> never split.
- EVERY transfer is DRAM-contiguous: w1 ships as three chunk tensors
  and w2 strictly as per-pair 256KB tiles. A strided source (2KB
  pieces at 8KB/256KB stride) drains at ~75-100 GB/s and, on a FIFO
  queue, stalls everything behind it (v9 trace).
- x streams as 16 single-row [128,1024] tiles (256KB completion
  granularity, row 0 lands ~10us); xp pool bufs=10 so x WAR stalls
  can't block the early w2/w1 transfers behind them.
- PE warmup: 12 contiguous full-array K=128 matmuls on a memset tile
  + 1 filler after each of rows 0-3. The HAM un-throttle needs a
  ~3.4us GAP-FREE aligned busy window -- a 92%-duty stream did NOT
  un-throttle for 20us (v5), and any PE gap >~2.5us re-throttles
  (v4). Real work then starts ~12.4us, warm.
- L1 PSUM evac was v3's hidden bottleneck: one-engine [128,1024] evacs
  (~1.1us) gated the 2-buffer PSUM recycle. v10 splits every L1 evac
  across ACT (cols 0:512, bank A) + DVE (512:1024, bank B) -- legal,
  parallel (verified in trace) -- with l1ps bufs=3 (6 banks; l2ps 2).
  l1ps=2/l2ps=4 measured 13us SLOWER.
- L2 emitted at full-chain granularity (8 k-steps, one position pair
  on PE col halves 0-63/64-127), interleaved with the L1 half-rows
  each pass unlocks: pass h = [chain, half-row, chain, ...]; when a
  pass has more half-rows than chains, a trailing chain splits into
  two 4-step chunks (two back-to-back half-rows stall 1-2us on the
  PSUM WAR; a chunk boundary ~0.1us).
- The Tile scheduler fine-weaves these units around genuinely-late
  dependencies (DVE evac halves, DMA arrivals); forcing strict unit
  order via wait_until pins measured 6us SLOWER (v9) -- the tick()
  pins left in place are inert end-floors.
- FC head: fcps bufs=4 + fc1 weights prefetched on the sync queue
  tail (bufs=3, WAR self-paced behind the FC1 chains) remove all
  inter-chain stalls; FC runs ~52us vs a 51us round floor. FC3
  interleaves into the FC2 chain loop; single-DVE [10,512] final copy
  (same-bank ACT+DVE splits serialize).
"""
import sys

if '/opt/trn_rl_repo' not in sys.path:
    sys.path.insert(0, '/opt/trn_rl_repo')

import numpy as np

N_CORES = 8
BS = 512
WARM_N = 12
LAST_EXEC_NS = None

# ----------------------------------------------------------------- host prep

def _prep_x(x):
    """x [B,3,32,32] -> [N_CORES, 16, 128, 1024] f16 row tiles.

    part = 32*i + 16*q + f; pair p=4g+i covers w1 in {2p,2p+1}; q = w1
    parity; f = c*4 + kh*2 + kw (12..15 zero-pad). Free dim = (g, batch).
    """
    ncr = x.shape[0] // BS
    xr = x.reshape(ncr, BS, 3, 16, 2, 2, 4, 2, 2)   # s,b,c,r,kh,g,i,q,kw
    xt = xr.transpose(0, 3, 5, 6, 7, 2, 4, 8, 1)    # s,r,g,i,q,c,kh,kw,b
    xt = xt.reshape(ncr, 16, 2, 4, 2, 12, BS)
    xpp = np.zeros((ncr, 16, 2, 4, 2, 16, BS), np.float16)
    xpp[..., :12, :] = xt
    # -> s, r, (i,q,f)=128, (g,b)=1024
    xpp = xpp.reshape(ncr, 16, 2, 128, BS).transpose(0, 1, 3, 2, 4)
    return np.ascontiguousarray(xpp.reshape(ncr, 16, 128, 1024))


def _prep_w1(conv1w):
    """conv1w [64,256,3,2,2] -> [128, 16*256] f16 block-diag strips.

    [p, r*256 + g*128 + c]: strip part p = 32i+16qp+f holds, for parity
    qp, features f -> out channel block c = 64*q + o with q==qp.
    """
    w1r = conv1w.reshape(64, 16, 16, 3, 2, 2)
    wt = w1r.transpose(1, 2, 3, 4, 5, 0).reshape(16, 16, 12, 64)
    wtp = np.zeros((16, 16, 16, 64), np.float32)
    wtp[:, :, :12, :] = wt
    wtp = wtp.reshape(16, 2, 4, 2, 16, 64)          # r,g,i,qp,f,o
    w1t = np.zeros((16, 2, 4, 2, 16, 2, 64), np.float32)
    w1t[:, :, :, 0, :, 0, :] = wtp[:, :, :, 0, :, :]
    w1t[:, :, :, 1, :, 1, :] = wtp[:, :, :, 1, :, :]
    w1t = w1t.reshape(16, 2, 128, 128)              # r,g,p,c
    w1t = w1t.transpose(2, 0, 1, 3)                 # p,r,g,c
    full = w1t.reshape(128, 16 * 256).astype(np.float16)
    # three DRAM-contiguous chunks in need order (rows 0-3 / 4-9 / 10-15):
    # a strided [:, lo:hi] slice of one [128,4096] tensor drains at
    # ~75-100 GB/s (2KB pieces at 8KB stride) and stalls the FIFO queue.
    return (np.ascontiguousarray(full[:, 0:1024]),
            np.ascontiguousarray(full[:, 1024:2560]),
            np.ascontiguousarray(full[:, 2560:4096]))


def _h2_posmap():
    pm = np.full((25, 2), -1, np.int64)
    for T in range(21):
        rr, j = divmod(T, 3)
        pm[T, 0] = rr * 7 + 2 * j
        pm[T, 1] = rr * 7 + 2 * j + 1
    for pi in range(4):
        r0, r1 = 2 * pi, 2 * pi + 1
        pm[21 + pi, 0] = r0 * 7 + 6
        if r1 < 7:
            pm[21 + pi, 1] = r1 * 7 + 6
    return pm


# pair-tile consumption order: pass h emits pairs [3h, 3h+1, 3h+2] plus
# cross pairs 21/22/23+24 at passes 2/4/6; w2 DRAM tiles are stored in
# this exact order so each pass is one contiguous DMA.
_W2_ORDER = [0, 1, 2, 3, 4, 5, 6, 7, 8, 21, 9, 10, 11, 12, 13, 14, 22,
             15, 16, 17, 18, 19, 20, 23, 24]
_W2_SLOT = {T: s for s, T in enumerate(_W2_ORDER)}


def _prep_w2(conv2w):
    """conv2w [64,49,64,4,4] -> [25, 128, 1024] f16 pair tiles in
    consumption (_W2_ORDER) order.

    Per position: [128=(q,c), 512=(kh,t,o)]; pair tile free dim =
    (member u, 512).
    """
    w2r = conv2w.reshape(64, 7, 7, 64, 4, 4)
    v = w2r.transpose(1, 2, 3, 4, 5, 0)             # h,w,c,kh,kw,o
    v = v.reshape(7, 7, 64, 4, 2, 2, 64)            # h,w,c,kh,t,q,o
    v = v.transpose(0, 1, 5, 2, 3, 4, 6)            # h,w,q,c,kh,t,o
    pos = v.reshape(49, 128, 512)
    pm = _h2_posmap()
    out = np.zeros((25, 128, 1024), np.float16)
    for T in range(25):
        s = _W2_SLOT[T]
        out[s, :, 0:512] = pos[pm[T, 0]]
        if pm[T, 1] >= 0:
            out[s, :, 512:1024] = pos[pm[T, 1]]
    return np.ascontiguousarray(out)


def _prep_fc1(fc1):
    """fc1 [1024, 3136] -> [8, 128, 3200] f16, k in h2-tile (T) order."""
    pm = _h2_posmap()
    fc1p = fc1.reshape(1024, 64, 49)
    fc1hat = np.zeros((1024, 25, 2, 64), np.float32)
    for T in range(25):
        for u in range(2):
            p = pm[T, u]
            if p >= 0:
                fc1hat[:, T, u, :] = fc1p[:, :, p]
    a = fc1hat.reshape(8, 128, 25, 128).transpose(0, 3, 2, 1)   # m,kp,k,mc
    return np.ascontiguousarray(a.reshape(8, 128, 3200)).astype(np.float16)


def _prep_fc2(fc2):
    """fc2 [512, 1024] -> [128, 4096] f16: [kp, (m k mc)]."""
    a = fc2.reshape(4, 128, 8, 128)                 # m,mc,k,kp
    a = a.transpose(3, 0, 2, 1)                     # kp,m,k,mc
    return np.ascontiguousarray(a.reshape(128, 4096)).astype(np.float16)


def _prep_fc3(fc3):
    """fc3 [10, 512] -> [128, 40] f16: [kp, (k o)]."""
    a = fc3.T.reshape(4, 128, 10)                   # k,kp,o
    a = a.transpose(1, 0, 2)                        # kp,k,o
    return np.ascontiguousarray(a.reshape(128, 40)).astype(np.float16)


# --------------------------------------------------------------- bass kernel

_NC_CACHE = []


def _build_nc():
    import concourse.bass as bass
    import concourse.mybir as mybir
    from concourse import bacc
    from concourse.tile import TileContext

    f32 = mybir.dt.float32
    f16 = mybir.dt.float16
    RELU = mybir.ActivationFunctionType.Relu

    nc = bacc.Bacc("TRN2", target_bir_lowering=False, debug=False,
                   num_devices=N_CORES)
    x_pp = nc.dram_tensor("x_pp", [16, 128, 1024], f16, kind="ExternalInput")
    w1a = nc.dram_tensor("w1a", [128, 1024], f16, kind="ExternalInput")
    w1b = nc.dram_tensor("w1b", [128, 1536], f16, kind="ExternalInput")
    w1c = nc.dram_tensor("w1c", [128, 1536], f16, kind="ExternalInput")
    w2t = nc.dram_tensor("w2t", [25, 128, 1024], f16, kind="ExternalInput")
    fc1m = nc.dram_tensor("fc1m", [8, 128, 3200], f16, kind="ExternalInput")
    fc2t = nc.dram_tensor("fc2t", [128, 4096], f16, kind="ExternalInput")
    fc3t = nc.dram_tensor("fc3t", [128, 40], f16, kind="ExternalInput")
    y = nc.dram_tensor("y", [10, 512], f32, kind="ExternalOutput")

    pm = _h2_posmap()
    # pass h -> pair tiles, in chain order
    pass_pairs = {h: [3 * h + j for j in range(3)] for h in range(7)}
    pass_pairs[2].append(21)
    pass_pairs[4].append(22)
    pass_pairs[6].extend([23, 24])

    ectr = [0]

    with TileContext(nc) as tc:
        with (
            tc.tile_pool(name="h2pool", bufs=25) as h2pool,
            tc.tile_pool(name="wpool", bufs=4) as wpool,
        ):
            h2 = [h2pool.tile([128, 512], f16, tag="h2", name=f"h2_{T}")
                  for T in range(25)]

            def relu_evac(dst, src):
                if ectr[0] % 2 == 0:
                    nc.scalar.activation(dst, src, RELU)
                else:
                    nc.vector.tensor_scalar_max(dst, src, 0.0)
                ectr[0] += 1

            # The warm-tile memset rides GpSimd: that queue carries
            # nothing else, so the tile is ready the moment the engine
            # preambles finish and the first warm matmul can issue.
            warm = wpool.tile([128, 512], f16, tag="warm", name="warm",
                              bufs=1)
            nc.gpsimd.memset(warm[:], 0.0)

            # fc1 weights stream from the sync queue tail (after all
            # phase-1 transfers); bufs=3 WAR stalls at the queue head
            # self-pace m3..m7 behind the FC1 chains that free their
            # buffers. Nothing later on the queue except y-out.
            fc1w = [None] * 8

            def load_fc1(m):
                wt = wpool.tile([128, 3200], f16, tag="fc1w",
                                name=f"fc1w_{m}", bufs=3)
                nc.sync.dma_start(out=wt[:], in_=fc1m.ap()[m])
                fc1w[m] = wt

            # ---------------- phase 1: L1 + L2 interleaved ----------------
            with (
                tc.tile_pool(name="xp", bufs=10) as xp_pool,
                tc.tile_pool(name="w1p", bufs=1) as w1_pool,
                tc.tile_pool(name="w2p", bufs=3) as w2_pool,
                tc.tile_pool(name="o1p", bufs=32) as o1_pool,
                tc.tile_pool(name="l1ps", bufs=3, space="PSUM") as l1ps,
                tc.tile_pool(name="l2ps", bufs=2, space="PSUM") as l2ps,
            ):
                xt = [None] * 16

                def load_x(r):
                    t = xp_pool.tile([128, 1024], f16, tag="xp",
                                     name=f"xp_{r}")
                    nc.sync.dma_start(out=t[:], in_=x_pp.ap()[r])
                    xt[r] = t

                w1tile = w1_pool.tile([128, 4096], f16, tag="w1",
                                      name="w1")

                _w1chunks = [(w1a, 0, 1024), (w1b, 1024, 2560),
                             (w1c, 2560, 4096)]

                def load_w1(ci):
                    src, lo, hi = _w1chunks[ci]
                    nc.sync.dma_start(out=w1tile[:, lo:hi],
                                      in_=src.ap())

                # w2 tiles: pass 0 as per-pair 256KB DMAs (fine-grain
                # early arrival), passes 1-6 as whole-pass DMAs.
                w2tiles = {}

                def load_w2_pair(T, w2tl, j):
                    s = _W2_SLOT[T]
                    nc.sync.dma_start(
                        out=w2tl[:, 1024 * j:1024 * j + 1024],
                        in_=w2t.ap()[s])
                    w2tiles[T] = w2tl[:, 1024 * j:1024 * j + 1024]

                def alloc_w2(h):
                    n = len(pass_pairs[h])
                    return w2_pool.tile([128, 1024 * n], f16, tag="w2",
                                        name=f"w2p_{h}", bufs=3)

                def load_w2_pass(h):
                    # per-pair 256KB DMAs: each is DRAM-contiguous; a
                    # whole-pass gathered transfer (2KB pieces at 256KB
                    # stride) drains ~4x slower and stalls the queue.
                    t = alloc_w2(h)
                    for j, T in enumerate(pass_pairs[h]):
                        load_w2_pair(T, t, j)

                # ---- sync-queue DMA emission, exact first-need order.
                # Only fresh pool buffers here (10 x tiles, 3 w2 slots,
                # the single w1 tile); recycled buffers are emitted
                # later, after their previous tenant's readers, so the
                # emission-order WAR tracking stays sound.
                load_x(0)
                load_w1(0)                  # rows 0-3
                load_x(1)
                load_x(2)
                w2t0 = alloc_w2(0)
                load_w2_pair(0, w2t0, 0)
                load_x(3)
                load_w2_pair(1, w2t0, 1)
                load_w2_pair(2, w2t0, 2)
                load_x(4)
                load_w1(1)                  # rows 4-9
                load_x(5)
                w2t1 = alloc_w2(1)
                load_w2_pair(3, w2t1, 0)
                load_x(6)
                load_w2_pair(4, w2t1, 1)
                load_x(7)
                load_w2_pair(5, w2t1, 2)
                load_x(8)
                load_w1(2)                  # rows 10-15
                load_x(9)
                load_w2_pass(2)

                # PE warmup: full-array (K=128, M=128) matmuls on the
                # memset tile so HAM un-throttles during the DMA ramp.
                wps = l2ps.tile([128, 512], f32, tag="l2", name="warm_ps")

                def emit_warm(n):
                    for _ in range(n):
                        nc.tensor.matmul(wps[:], warm[:, 0:128], warm[:],
                                         start=True, stop=True)

                out1 = [[None] * 8 for _ in range(16)]

                def emit_l1_half_unpinned(r, g):
                    # 2 half-groups of 2 concurrent K=32 strip matmuls
                    # (tile_position row groups); each (g, half) lands
                    # in a [128,1024] PSUM tile whose evac is split
                    # ACT (cols 0:512, bank A) / DVE (512:1024, bank B)
                    # to halve the recycle latency.
                    w1row = w1tile[:, 256 * r:256 * r + 256]
                    for half in range(2):
                        ps = l1ps.tile([128, 1024], f32, tag="l1",
                                       name=f"l1ps_{r}_{g}_{half}")
                        for sub in range(2):
                            i = half + 2 * sub
                            nc.tensor.matmul(
                                ps[:, 512 * sub:512 * sub + 512],
                                w1row[32 * i:32 * i + 32,
                                      128 * g:128 * g + 128],
                                xt[r][32 * i:32 * i + 32,
                                      512 * g:512 * g + 512],
                                start=True, stop=True,
                                tile_position=(32 * i, 0))
                        ot = o1_pool.tile([128, 1024], f16, tag="o1",
                                          name=f"o1_{r}_{g}_{half}")
                        nc.scalar.activation(ot[:, 0:512],
                                             ps[:, 0:512], RELU)
                        nc.vector.tensor_scalar_max(ot[:, 512:1024],
                                                    ps[:, 512:1024],
                                                    0.0)
                        for sub in range(2):
                            out1[r][4 * g + half + 2 * sub] = \
                                ot[:, 512 * sub:512 * sub + 512]

                def emit_l1_half(r, g):
                    with tick(0.00045):
                        emit_l1_half_unpinned(r, g)

                def emit_l1_row(r):
                    emit_l1_half(r, 0)
                    emit_l1_half(r, 1)

                def emit_chain_part(T, k0, k1, ps):
                    # One position pair: A/B chains on PE col strips
                    # 0-63 / 64-127 share one [128,512] PSUM tile split
                    # by partition range; one evac covers the pair.
                    pA, pB = pm[T]
                    hA, wA = divmod(int(pA), 7)
                    hB, wB = (None, None) if pB < 0 else divmod(int(pB), 7)
                    wt2 = w2tiles[T]
                    if ps is None:
                        ps = l2ps.tile([128, 512], f32, tag="l2",
                                       name=f"l2ps_{T}")
                    for kt in range(k0, k1):
                        kh, t = divmod(kt, 2)
                        nc.tensor.matmul(
                            ps[0:64, :],
                            wt2[:, 64 * kt:64 * kt + 64],
                            out1[2 * hA + kh][wA + t],
                            start=(kt == 0), stop=(kt == 7),
                            tile_position=(0, 0))
                        if hB is not None:
                            nc.tensor.matmul(
                                ps[64:128, :],
                                wt2[:, 512 + 64 * kt:512 + 64 * kt + 64],
                                out1[2 * hB + kh][wB + t],
                                start=(kt == 0), stop=(kt == 7),
                                tile_position=(0, 64))
                    if k1 == 8:
                        relu_evac(h2[T][:], ps[:])
                    return ps

                def emit_chain_part_pinned(T, k0, k1, ps):
                    with tick(0.0019 * (k1 - k0) / 8):
                        return emit_chain_part(T, k0, k1, ps)

                def emit_chain(T):
                    emit_chain_part_pinned(T, 0, 8, None)

                def run_pass(h, rows):
                    # interleave at half-row granularity: a chain (or
                    # half-chain) between any two L1 half-rows gives the
                    # l1ps recycle (gated by the laggy DVE evac queue)
                    # enough slack. When a pass has more half-rows than
                    # chains, SPLIT trailing chains into two 4-step
                    # chunks -- two back-to-back half-rows stall ~1-2us
                    # on the PSUM WAR (v6 trace), a chunk boundary only
                    # ~0.1us.
                    ts = pass_pairs[h]
                    halves = [(r, g) for r in rows for g in range(2)]
                    n_extra = max(0, len(halves) - len(ts))
                    hi = 0

                    def next_half():
                        nonlocal hi
                        if hi < len(halves):
                            emit_l1_half(*halves[hi])
                            hi += 1

                    for j, T in enumerate(ts):
                        if j >= len(ts) - n_extra:
                            ps = emit_chain_part_pinned(T, 0, 4, None)
                            next_half()
                            emit_chain_part_pinned(T, 4, 8, ps)
                        else:
                            emit_chain(T)
                        next_half()

                # ---- PE emission: warmups + rows 0-3, then passes.
                # Recycled-buffer DMAs are emitted at the first point
                # after their WAR predecessor's readers; their queue
                # position still honors first-need order.
                #
                # Phase-1 PE units are PINNED to a paced logical clock
                # (tile_wait_until gates only the Tile scheduler's sim,
                # not runtime): without pins the scheduler weaves L1
                # half-rows INTO chain-round gaps and even splits the
                # A/B col-pair of one k-step (v7 trace, 15-21us: ~320ns
                # rounds from per-switch weight-reload exposure).
                # NOTE: forcing strict unit order via start-floor pins
                # measured 6us SLOWER (v9) -- the scheduler's fine
                # interleave works around genuinely-late dependencies
                # (DVE evac halves, DMA arrivals) better than a rigid
                # order. Keep pins as inert end-floors (no-ops in
                # practice; v8 == v7 within noise).
                pin = [0.004]

                def tick(d):
                    pin[0] += d
                    return tc.tile_wait_until(pin[0])

                emit_warm(WARM_N)
                emit_l1_row(0)
                load_x(10)
                emit_warm(2)
                emit_l1_row(1)
                load_x(11)
                emit_warm(2)
                emit_l1_row(2)
                load_x(12)
                emit_warm(1)
                emit_l1_row(3)
                load_x(13)
                emit_warm(1)
                run_pass(0, [4, 5])
                load_x(14)
                load_x(15)
                load_w2_pass(3)
                run_pass(1, [6, 7])
                load_w2_pass(4)
                run_pass(2, [8, 9])
                load_w2_pass(5)
                run_pass(3, [10, 11])
                load_w2_pass(6)
                fc2w = wpool.tile([128, 4096], f16, tag="fc2w",
                                  name="fc2w", bufs=1)
                nc.sync.dma_start(out=fc2w[:], in_=fc2t.ap())
                fc3w = wpool.tile([128, 40], f16, tag="fc3w",
                                  name="fc3w", bufs=1)
                nc.sync.dma_start(out=fc3w[:], in_=fc3t.ap())
                load_fc1(0)
                load_fc1(1)
                load_fc1(2)
                run_pass(4, [12, 13])
                run_pass(5, [14, 15])
                run_pass(6, [])

            # ---------------- phase 2: FC head ----------------
            with (
                tc.tile_pool(name="fcio", bufs=12) as fcio_pool,
                tc.tile_pool(name="fcps", bufs=4, space="PSUM") as fcps,
                tc.tile_pool(name="fc3ps", bufs=1, space="PSUM") as fc3ps,
            ):
                h3 = []
                for m in range(8):
                    wt = fc1w[m]
                    ps = fcps.tile([128, 512], f32, tag="fc",
                                   name=f"fc1ps_{m}")
                    for k in range(25):
                        nc.tensor.matmul(ps[:],
                                         wt[:, 128 * k:128 * k + 128],
                                         h2[k][:],
                                         start=(k == 0), stop=(k == 24))
                    ot = fcio_pool.tile([128, 512], f16, tag="h3",
                                        name=f"h3_{m}", bufs=8)
                    relu_evac(ot[:], ps[:])
                    h3.append(ot)
                    if m < 5:
                        load_fc1(m + 3)

                # FC3 accumulates k-major into one [10, 512] PSUM chain
                # (out = fc3.T slice as lhsT, h4[k] moving), interleaved
                # into the FC2 chain loop; output is y [10, 512], the
                # host transposes back to [512, 10].
                h4 = []
                ps3 = fc3ps.tile([128, 512], f32, tag="fc3", name="fc3ps")

                for m in range(4):
                    ps = fcps.tile([128, 512], f32, tag="fc",
                                   name=f"fc2ps_{m}")
                    for k in range(8):
                        nc.tensor.matmul(
                            ps[:],
                            fc2w[:, 1024 * m + 128 * k:
                                 1024 * m + 128 * k + 128],
                            h3[k][:],
                            start=(k == 0), stop=(k == 7))
                    ot = fcio_pool.tile([128, 512], f16, tag="h4",
                                        name=f"h4_{m}", bufs=4)
                    if m == 3:
                        # same-bank ACT+DVE splits serialize (v4 trace)
                        # -- single ACT op is the fastest evac here.
                        nc.scalar.activation(ot[:], ps[:], RELU)
                    else:
                        relu_evac(ot[:], ps[:])
                    h4.append(ot)
                    if m >= 1:
                        nc.tensor.matmul(
                            ps3[0:10, :], fc3w[:, 10 * (m - 1):10 * m],
                            h4[m - 1][:],
                            start=(m == 1), stop=False)
                nc.tensor.matmul(ps3[0:10, :], fc3w[:, 30:40], h4[3][:],
                                 start=False, stop=True)

                yt = fcio_pool.tile([128, 512], f32, tag="yt", name="yt",
                                    bufs=1)
                nc.vector.tensor_copy(yt[0:10, :], ps3[0:10, :])
                nc.sync.dma_start(out=y.ap()[:], in_=yt[0:10, :])
    nc.compile()
    return nc


def kernel(x, conv1w, conv2w, fc1, fc2, fc3):
    global LAST_EXEC_NS
    from concourse.bass_utils import run_bass_kernel_spmd

    x = np.ascontiguousarray(np.asarray(x, dtype=np.float32))
    conv1w = np.ascontiguousarray(np.asarray(conv1w, dtype=np.float32))
    conv2w = np.ascontiguousarray(np.asarray(conv2w, dtype=np.float32))
    fc1 = np.ascontiguousarray(np.asarray(fc1, dtype=np.float32))
    fc2 = np.ascontiguousarray(np.asarray(fc2, dtype=np.float32))
    fc3 = np.ascontiguousarray(np.asarray(fc3, dtype=np.float32))

    if not _NC_CACHE:
        _NC_CACHE.append(_build_nc())
    nc = _NC_CACHE[0]

    xpp = _prep_x(x.astype(np.float16))
    w1abc = _prep_w1(conv1w)
    shared = {
        "w1a": w1abc[0], "w1b": w1abc[1], "w1c": w1abc[2],
        "w2t": _prep_w2(conv2w),
        "fc1m": _prep_fc1(fc1),
        "fc2t": _prep_fc2(fc2),
        "fc3t": _prep_fc3(fc3),
    }
    in_maps = [{**shared, "x_pp": xpp[c]} for c in range(N_CORES)]
    res = run_bass_kernel_spmd(nc, in_maps, list(range(N_CORES)))
    LAST_EXEC_NS = res.exec_time_ns
    # y is [10, 512] per core -> [512, 10]
    outs = [np.ascontiguousarray(r["y"].T) for r in res.results]
    return np.ascontiguousarray(np.concatenate(outs, axis=0))
